# revision 2
# baseline (speedup 1.0000x reference)
"""Trainium2 Bass kernel for nn_EquivariantScalar_viaTP — V3.3.

Reference computation (after dead-code elimination — the gate / l=1 / l=2
paths never reach the output):

    s      = node_vec[:, :128]                                  # [N, 128]
    attr   = node_embedding                                     # [N, 32]
    s_mid  = einsum('nu,nv,uvw->nw', s, attr, W1s) / 64 + b1s   # [N, 128]
    s_act  = silu(s_mid)
    h      = einsum('nu,nv,uvw->nw', s_act, attr, W2) / 64 + b2 # [N, 32]
    h      = silu(h @ (W3/sqrt(32)) + b3)                       # [N, 32]
    out    = h @ (W4/sqrt(32)) + b4                             # [N, 1]

Sharding: node dim N=8192 across 8 cores (1024 nodes each).

V3 design (engine-balanced, cost-model driven):
  Stage 1 — Z-outer-product form: s_mid^T[w,n] = sum_k W1f[k,w] Z[k,n],
  k=(u,v), 32 accumulating bf16 k-tile matmuls per 256-node block.
  k-tile tau=(tv,tu): partition p maps u = 32*tu + p%32, v = 4*tv + p//32.
  Z is formed elementwise from replicated sT / attrT tiles; broadcasts on
  the two OUTER free dims keep the 2x bf16 DVE mode.  The DVE forms
  tv 0..6 (two fused ops), the otherwise-idle Pool engine forms tv 7.
  silu(+b1) -> sact^T [u,n] bf16.

  Stage 2 transposed (T2T): per half-block, 4 matmuls with lhsT = W2f_b
  [128u, 128(v,w)] and rhs = sact^T produce T2T[(v,w), n] in PSUM
  (v(p2)=4b+p2//32, w(p2)=p2%32).  The attr multiply in2 is the SAME sbc
  attr tile stage 1 streams (its partition map matches v(p2) by
  construction).  GPSIMD cannot read PSUM (walrus rejects it), so blocks
  0..1 route ACT-copy(PSUM->SBUF bf16) -> Pool multiply, and blocks 2..3
  (DVE idle by then) multiply on the DVE straight from PSUM.
  The v-contraction is 8 accumulating PE matmuls per 128-node chunk with
  lhsT = a 0/1 selector Sel[p2,w] = (p2%32==w) and rhs = P2 — h2 lands
  TRANSPOSED [32w, n] in PSUM, so stages 3/4 need no PE transposes at
  all.  b2 folds into b3 (b3_eff = b3 + b2 @ W3/sqrt(A)).

  Stages 3/4 per 512-node group, all in the [feature, node] orientation:
  one ACT copy (PSUM->SBUF bf16), matmul lhsT=W3n, silu with per-
  partition bias b3, matmul lhsT=W4n, biased Identity copy -> out row
  [1, 512].  Output is [1, 1024] per core, node-ordered.

  The PE stream is emitted interleaved (S1(i) | T2T(i-1) | Sel(i-2)) and
  a tunable dummy-matmul warm-up bridges the PE through the DMA fill so
  real matmuls dispatch into a fully ramped p-state (the cost model
  charges p-state at dispatch; idle resets the ramp).
"""

import os

import numpy as np
import ml_dtypes

import concourse.bass as bass
import concourse.bacc as bacc
import concourse.mybir as mybir
from concourse.tile import TileContext
from concourse import bass_utils

N = 8192
P = 128          # partitions / MUL0
A = 32           # attr channels
NCORES = 8
NPC = N // NCORES          # 1024 nodes per core
NCHUNK = NPC // P          # 8 chunks per core
NBLK = 8                   # node blocks per core
BLK = NPC // NBLK          # 128 nodes
GS = 4                     # s-side tiles per block   (u = 32*tu + p%32)
GA = 8                     # attr-side tiles per block (v = 4*tv + p//32)
KT = GS * GA               # 32 k-tiles
HGA = GA // 2              # 4 attr tiles per half
SCOLS = GS * BLK           # 1024 s cols per block
ACOLS = HGA * BLK          # 1024 attr cols per half
PIECE = SCOLS + 2 * ACOLS  # 3072
NGRP = 8                   # stage-3/4 groups
GRP = NPC // NGRP          # 256 nodes per group
# per-half mult routing: 16 halves (2 per block)
MULT_ROUTE = ["pool", "dve2x"] * 7 + ["dve", "dve"]
WARMUP_MM = 66

F32 = mybir.dt.float32
BF16 = mybir.dt.bfloat16
BF = ml_dtypes.bfloat16

# wconst (bf16) column offsets
OFF_W1 = 0
OFF_W2 = OFF_W1 + KT * P          # 4096
OFF_SEL = OFF_W2 + GA * P         # 5120
OFF_W3 = OFF_SEL + A              # w3n on partitions 0..31
OFF_W4 = OFF_W3 + A               # w4n on partitions 0..31
FWC = OFF_W4 + 1                  # 5185
FCC = 3                           # cconst (f32): b1col | b3col | b4col


_CACHE = {}
LAST_RESULT = None         # test harness reads exec_time_ns from here


def _build():
    nc = bacc.Bacc(trn_type="TRN2", target_bir_lowering=False, debug=False)

    wconst_d = nc.dram_tensor("wconst", [P, FWC], BF16, kind="ExternalInput")
    cconst_d = nc.dram_tensor("cconst", [P, FCC], F32, kind="ExternalInput")
    sbc_d = nc.dram_tensor("sbc", [P, NBLK * PIECE], BF16,
                           kind="ExternalInput")
    out_d = nc.dram_tensor("out", [1, NPC], F32, kind="ExternalOutput")

    Alu = mybir.AluOpType
    Act = mybir.ActivationFunctionType

    with TileContext(nc) as tc:
        with (
            tc.tile_pool(name="const", bufs=1) as cp,
            tc.tile_pool(name="pc", bufs=6) as pc_p,
            tc.tile_pool(name="za", bufs=2) as za_p,
            tc.tile_pool(name="zb", bufs=2) as zb_p,
            tc.tile_pool(name="sact", bufs=3) as sact_p,
            tc.tile_pool(name="t2c", bufs=3) as t2c_p,
            tc.tile_pool(name="p2", bufs=8) as p2_p,
            tc.tile_pool(name="h2t", bufs=2) as h2t_p,
            tc.tile_pool(name="h3t", bufs=2) as h3t_p,
            tc.tile_pool(name="psacc", bufs=2, space="PSUM") as ps_acc,
            tc.tile_pool(name="pst2", bufs=4, space="PSUM") as ps_t2,
            tc.tile_pool(name="psh2", bufs=2, space="PSUM") as ps_h2,
        ):
            wconst = cp.tile([P, FWC], BF16, tag="wconst")
            cconst = cp.tile([P, FCC], F32, tag="cconst")
            pcs = {}

            def dma_sbc(q, split=False):
                pc = pc_p.tile([P, PIECE], BF16, tag="pc", name=f"pc{q}")
                if split:
                    nc.sync.dma_start(
                        pc[:, 0:SCOLS + ACOLS],
                        sbc_d.ap()[:, q * PIECE:q * PIECE + SCOLS + ACOLS])
                    nc.sync.dma_start(
                        pc[:, SCOLS + ACOLS:],
                        sbc_d.ap()[:, q * PIECE + SCOLS + ACOLS:
                                   (q + 1) * PIECE])
                else:
                    nc.sync.dma_start(
                        pc[:], sbc_d.ap()[:, q * PIECE:(q + 1) * PIECE])
                pcs[q] = pc

            # DMA dispatch order: SP ring carries sbc pieces; ACT ring
            # carries w1 (quarters, so S1(0) unblocks progressively), the
            # rest of wconst, and cconst.  Dispatch interleaves so the
            # shared wire serves the z-critical pieces first.
            WQ = KT * P // 4
            dma_sbc(0, split=True)
            nc.scalar.dma_start(wconst[:, 0:WQ], wconst_d.ap()[:, 0:WQ])
            dma_sbc(1, split=True)
            nc.scalar.dma_start(wconst[:, WQ:2 * WQ],
                                wconst_d.ap()[:, WQ:2 * WQ])
            dma_sbc(2, split=True)
            nc.scalar.dma_start(wconst[:, 2 * WQ:3 * WQ],
                                wconst_d.ap()[:, 2 * WQ:3 * WQ])
            dma_sbc(3, split=True)
            nc.scalar.dma_start(wconst[:, 3 * WQ:4 * WQ],
                                wconst_d.ap()[:, 3 * WQ:4 * WQ])
            nc.scalar.dma_start(wconst[:, 4 * WQ:], wconst_d.ap()[:, 4 * WQ:])
            nc.scalar.dma_start(cconst[:], cconst_d.ap())
            for q in range(4, NBLK):
                dma_sbc(q, split=True)

            # ---- warm-up: hoist all activation-table loads to t~0 and
            # bridge the PE through the DMA fill with dummy matmuls so the
            # first real matmul dispatches into a ramped p-state.
            scr = cp.tile([P, 192], BF16, tag="scr")
            nc.gpsimd.memset(scr[:], 0.0)
            scrf = cp.tile([P, 3], F32, tag="scrf")
            nc.gpsimd.memset(scrf[:], 0.0)
            nc.scalar.activation(scrf[:, 1:2], scrf[:, 0:1], Act.Silu)
            nc.scalar.copy(scrf[:, 1:2], scrf[:, 0:1])
            nc.scalar.activation(scrf[:, 2:3], scrf[:, 0:1], Act.Identity,
                                 bias=0.0)
            pscr = ps_h2.tile([P, 64], F32, tag="h2ps", name="pscr")
            for _ in range(WARMUP_MM):
                nc.tensor.matmul(pscr[:], scr[:, 0:128], scr[:, 128:192],
                                 start=True, stop=True)

            def stile(q):
                return pcs[q][:, 0:SCOLS]

            def atiles(q):
                return (pcs[q][:, SCOLS:SCOLS + ACOLS],
                        pcs[q][:, SCOLS + ACOLS:])

            w1f = wconst[:, OFF_W1:OFF_W1 + KT * P]
            w2f = wconst[:, OFF_W2:OFF_W2 + GA * P]
            sel = wconst[:, OFF_SEL:OFF_SEL + A]
            w3n = wconst[0:A, OFF_W3:OFF_W3 + A]
            w4n = wconst[0:A, OFF_W4:OFF_W4 + 1]
            b1col = cconst[:, 0:1]
            b3col = cconst[0:A, 1:2]
            b4one = cconst[0:1, 2:3]

            outsb = cp.tile([1, NPC], F32, tag="outsb")

            zas, zbs, sacts, p2s, h2ps = {}, {}, {}, {}, {}

            def z_form(q, split0=False):
                """DVE: tv 0..3 (za) and tv 4..6 (zb); Pool: tv 7."""
                st, (aa, ab) = stile(q), atiles(q)
                s_v = st.rearrange("p (tu n) -> p tu n", n=BLK)
                a_va = aa.rearrange("p (tv n) -> p tv n", n=BLK)
                a_vb = ab.rearrange("p (tv n) -> p tv n", n=BLK)
                za = za_p.tile([P, 16 * BLK], BF16, tag="za", name=f"za{q}")
                zb = zb_p.tile([P, 16 * BLK], BF16, tag="zb", name=f"zb{q}")
                za_v = za[:].rearrange("p (tv tu n) -> p tv tu n",
                                       tu=GS, n=BLK)
                if split0:
                    for h in range(2):
                        nc.vector.tensor_tensor(
                            za_v[:, 2 * h:2 * h + 2],
                            s_v.unsqueeze(1).broadcast_to([P, 2, GS, BLK]),
                            a_va[:, 2 * h:2 * h + 2].unsqueeze(2)
                            .broadcast_to([P, 2, GS, BLK]),
                            Alu.mult)
                else:
                    nc.vector.tensor_tensor(
                        za_v,
                        s_v.unsqueeze(1).broadcast_to([P, 4, GS, BLK]),
                        a_va.unsqueeze(2).broadcast_to([P, 4, GS, BLK]),
                        Alu.mult)
                nc.vector.tensor_tensor(
                    zb[:, 0:12 * BLK].rearrange(
                        "p (tv tu n) -> p tv tu n", tu=GS, n=BLK),
                    s_v.unsqueeze(1).broadcast_to([P, 3, GS, BLK]),
                    a_vb[:, 0:3, :].unsqueeze(2).broadcast_to(
                        [P, 3, GS, BLK]),
                    Alu.mult)
                nc.gpsimd.tensor_tensor(
                    zb[:, 12 * BLK:].rearrange(
                        "p (tu n) -> p tu n", n=BLK),
                    s_v,
                    a_vb[:, 3:4, :].broadcast_to([P, GS, BLK]),
                    Alu.mult)
                zas[q], zbs[q] = za, zb

            def s1_matmuls(q):
                acc = ps_acc.tile([P, BLK], F32, tag="acc", name=f"acc{q}")
                for t in range(KT):
                    z = zas[q] if t < KT // 2 else zbs[q]
                    zc = (t % (KT // 2)) * BLK
                    nc.tensor.matmul(
                        acc[:], w1f[:, t * P:(t + 1) * P],
                        z[:, zc:zc + BLK],
                        start=(t == 0), stop=(t == KT - 1))
                sact = sact_p.tile([P, BLK], BF16, tag="sact",
                                   name=f"sact{q}")
                nc.scalar.activation(sact[:], acc[:], Act.Silu, bias=b1col)
                sacts[q] = sact

            def t2t_and_mult(q):
                """Per half: 4 T2T matmuls, then the attr multiply.
                Route 'pool': ACT copy PSUM->SBUF bf16, Pool multiplies.
                Route 'dve2x': ACT copy, DVE multiplies at the 2x rate.
                Route 'dve': DVE multiplies straight from PSUM (no copy —
                used only at the drain to shorten the last chain)."""
                aa, ab = atiles(q)
                outs = []
                for half, asrc in enumerate((aa, ab)):
                    route = MULT_ROUTE[2 * q + half]
                    t2 = ps_t2.tile([P, ACOLS], F32, tag="t2",
                                    name=f"t2_{q}_{half}")
                    for bb in range(HGA):
                        nc.tensor.matmul(
                            t2[:, bb * BLK:(bb + 1) * BLK],
                            w2f[:, (half * HGA + bb) * P:
                                (half * HGA + bb + 1) * P],
                            sacts[q][:], start=True, stop=True)
                    p2 = p2_p.tile([P, ACOLS], BF16, tag="p2",
                                   name=f"p2_{q}_{half}")
                    if route == "dve":
                        nc.vector.tensor_tensor(p2[:], t2[:], asrc,
                                                Alu.mult)
                    else:
                        t2c = t2c_p.tile([P, ACOLS], BF16, tag="t2c",
                                         name=f"t2c_{q}_{half}")
                        nc.scalar.copy(t2c[:], t2[:])
                        eng = nc.gpsimd if route == "pool" else nc.vector
                        eng.tensor_tensor(p2[:], t2c[:], asrc, Alu.mult)
                    outs.append(p2)
                p2s[q] = tuple(outs)

            def sel_matmuls(q):
                """block q (one 128-node chunk): 8 accumulating matmuls
                with lhsT = Sel -> h2ps[q] [32w, n] (transposed)."""
                h2ps[q] = ps_h2.tile([A, GRP], F32, tag="h2ps",
                                     name=f"h2ps{q}")
                p2a, p2b = p2s[q]
                dst = h2ps[q][:]
                for b in range(GA):
                    src = p2a if b < HGA else p2b
                    c0 = (b % HGA) * BLK
                    nc.tensor.matmul(dst, sel, src[:, c0:c0 + BLK],
                                     start=(b == 0), stop=(b == GA - 1))

            def s34(g):
                """stages 3/4 for the 256-node group g, [feature, node]."""
                h2t = h2t_p.tile([A, GRP], BF16, tag="h2t", name=f"h2t{g}")
                nc.scalar.copy(h2t[:], h2ps[g][:])
                o3 = ps_acc.tile([A, GRP], F32, tag="acc", name=f"o3{g}")
                nc.tensor.matmul(o3[:], w3n, h2t[:], start=True, stop=True)
                h3t = h3t_p.tile([A, GRP], BF16, tag="h3t", name=f"h3t{g}")
                nc.scalar.activation(h3t[:], o3[:], Act.Silu, bias=b3col)
                o4 = ps_acc.tile([1, GRP], F32, tag="acc", name=f"o4{g}")
                nc.tensor.matmul(o4[:], w4n, h3t[:], start=True, stop=True)
                nc.scalar.activation(outsb[:, g * GRP:(g + 1) * GRP], o4[:],
                                     Act.Identity, bias=b4one)
                nc.sync.dma_start(out_d.ap()[:, g * GRP:(g + 1) * GRP],
                                  outsb[:, g * GRP:(g + 1) * GRP])

            # ---- pipelined emission ----
            z_form(0, split0=True)
            z_form(1)
            for i in range(NBLK):
                if i + 2 < NBLK:
                    z_form(i + 2)
                s1_matmuls(i)
                if i >= 1:
                    t2t_and_mult(i - 1)
                if i == NBLK - 1:
                    t2t_and_mult(i)
                if i >= 3:
                    sel_matmuls(i - 3)
                    s34(i - 3)
            # tail
            sel_matmuls(NBLK - 3)
            s34(NBLK - 3)
            sel_matmuls(NBLK - 2)
            s34(NBLK - 2)
            sel_matmuls(NBLK - 1)
            s34(NBLK - 1)

    nc.compile()
    return nc


def _get_nc():
    if "nc" not in _CACHE:
        _CACHE["nc"] = _build()
    return _CACHE["nc"]


def _prep_inputs(node_vec, node_embedding, W1s, b1s, W2, b2, W3, b3, W4, b4):
    f = np.float32
    inv = f(1.0 / 64.0)                      # 1/sqrt(128*32)
    s = np.ascontiguousarray(node_vec[:, :P]).astype(f)
    attr = np.asarray(node_embedding, f)

    pidx = np.arange(P)
    # k-tile tau = 4*tv + tu:  u = 32*tu + p%32,  v = 4*tv + p//32
    su_rows = A * np.arange(GS)[:, None] + (pidx % A)[None, :]    # [GS, P]
    av_rows = GS * np.arange(GA)[:, None] + (pidx // A)[None, :]  # [GA, P]

    wconst = np.zeros((P, FWC), BF)
    w1 = (np.asarray(W1s, f) * inv).astype(BF)           # [128u, 32v, 128w]
    for tv in range(GA):
        for tu in range(GS):
            t = GS * tv + tu
            wconst[:, OFF_W1 + t * P:OFF_W1 + (t + 1) * P] = \
                w1[su_rows[tu], av_rows[tv], :]
    # W2f_b[u, p2] = W2[u, 4b + p2//32, p2%32] * inv
    w2 = np.asarray(W2, f) * inv                         # [128u, 32v, 32w]
    for b in range(GA):
        wconst[:, OFF_W2 + b * P:OFF_W2 + (b + 1) * P] = \
            w2[:, GS * b + pidx // A, pidx % A].astype(BF)
    wconst[:, OFF_SEL:OFF_SEL + A] = \
        (pidx[:, None] % A == np.arange(A)[None, :]).astype(BF)
    w3n = np.asarray(W3, f) / np.sqrt(f(A))
    w4n = np.asarray(W4, f) / np.sqrt(f(A))
    wconst[0:A, OFF_W3:OFF_W3 + A] = w3n.astype(BF)
    wconst[0:A, OFF_W4:OFF_W4 + 1] = w4n.astype(BF)

    b3_eff = np.asarray(b3, f) + np.asarray(b2, f) @ w3n
    cconst = np.zeros((P, FCC), f)
    cconst[:, 0] = np.asarray(b1s, f)
    cconst[0:A, 1] = b3_eff
    cconst[0, 2] = np.asarray(b4, f).reshape(-1)[0]

    in_maps = []
    for core in range(NCORES):
        lo = core * NPC
        S = s[lo:lo + NPC].astype(BF)                     # [1024, 128]
        atb = attr[lo:lo + NPC].astype(BF)                # [1024, 32]

        sbc = np.empty((P, NBLK * PIECE), BF)
        for q in range(NBLK):
            Sb = S[q * BLK:(q + 1) * BLK]                 # [BLK, 128]
            Ab = atb[q * BLK:(q + 1) * BLK]               # [BLK, 32]
            base = q * PIECE
            for g in range(GS):
                sbc[:, base + g * BLK: base + (g + 1) * BLK] = Sb.T[su_rows[g]]
            base += GS * BLK
            for g in range(GA):
                sbc[:, base + g * BLK: base + (g + 1) * BLK] = Ab.T[av_rows[g]]

        in_maps.append(dict(wconst=wconst, cconst=cconst, sbc=sbc))
    return in_maps


def kernel(**inputs):
    global LAST_RESULT
    trace = bool(int(os.environ.get("KERNEL_TRACE", "0")))
    in_maps = _prep_inputs(
        inputs["node_vec"], inputs["node_embedding"],
        inputs["W1s"], inputs["b1s"], inputs["W2"], inputs["b2"],
        inputs["W3"], inputs["b3"], inputs["W4"], inputs["b4"],
    )
    nc = _get_nc()
    res = bass_utils.run_bass_kernel_spmd(
        nc, in_maps, core_ids=list(range(NCORES)), trace=trace)
    LAST_RESULT = res
    outs = [np.asarray(res.results[i]["out"]) for i in range(NCORES)]
    energy = np.concatenate([o.reshape(NPC) for o in outs]).reshape(N, 1)
    return energy.astype(np.float32)


# revision 3
# speedup vs baseline: 1.0045x; 1.0045x over previous
"""Trainium2 Bass kernel for nn_EquivariantScalar_viaTP — V3.3.

Reference computation (after dead-code elimination — the gate / l=1 / l=2
paths never reach the output):

    s      = node_vec[:, :128]                                  # [N, 128]
    attr   = node_embedding                                     # [N, 32]
    s_mid  = einsum('nu,nv,uvw->nw', s, attr, W1s) / 64 + b1s   # [N, 128]
    s_act  = silu(s_mid)
    h      = einsum('nu,nv,uvw->nw', s_act, attr, W2) / 64 + b2 # [N, 32]
    h      = silu(h @ (W3/sqrt(32)) + b3)                       # [N, 32]
    out    = h @ (W4/sqrt(32)) + b4                             # [N, 1]

Sharding: node dim N=8192 across 8 cores (1024 nodes each).

V3 design (engine-balanced, cost-model driven):
  Stage 1 — Z-outer-product form: s_mid^T[w,n] = sum_k W1f[k,w] Z[k,n],
  k=(u,v), 32 accumulating bf16 k-tile matmuls per 256-node block.
  k-tile tau=(tv,tu): partition p maps u = 32*tu + p%32, v = 4*tv + p//32.
  Z is formed elementwise from replicated sT / attrT tiles; broadcasts on
  the two OUTER free dims keep the 2x bf16 DVE mode.  The DVE forms
  tv 0..6 (two fused ops), the otherwise-idle Pool engine forms tv 7.
  silu(+b1) -> sact^T [u,n] bf16.

  Stage 2 transposed (T2T): per half-block, 4 matmuls with lhsT = W2f_b
  [128u, 128(v,w)] and rhs = sact^T produce T2T[(v,w), n] in PSUM
  (v(p2)=4b+p2//32, w(p2)=p2%32).  The attr multiply in2 is the SAME sbc
  attr tile stage 1 streams (its partition map matches v(p2) by
  construction).  GPSIMD cannot read PSUM (walrus rejects it), so blocks
  0..1 route ACT-copy(PSUM->SBUF bf16) -> Pool multiply, and blocks 2..3
  (DVE idle by then) multiply on the DVE straight from PSUM.
  The v-contraction is 8 accumulating PE matmuls per 128-node chunk with
  lhsT = a 0/1 selector Sel[p2,w] = (p2%32==w) and rhs = P2 — h2 lands
  TRANSPOSED [32w, n] in PSUM, so stages 3/4 need no PE transposes at
  all.  b2 folds into b3 (b3_eff = b3 + b2 @ W3/sqrt(A)).

  Stages 3/4 per 512-node group, all in the [feature, node] orientation:
  one ACT copy (PSUM->SBUF bf16), matmul lhsT=W3n, silu with per-
  partition bias b3, matmul lhsT=W4n, biased Identity copy -> out row
  [1, 512].  Output is [1, 1024] per core, node-ordered.

  The PE stream is emitted interleaved (S1(i) | T2T(i-1) | Sel(i-2)) and
  a tunable dummy-matmul warm-up bridges the PE through the DMA fill so
  real matmuls dispatch into a fully ramped p-state (the cost model
  charges p-state at dispatch; idle resets the ramp).
"""

import os

import numpy as np
import ml_dtypes

import concourse.bass as bass
import concourse.bacc as bacc
import concourse.mybir as mybir
from concourse.tile import TileContext
from concourse import bass_utils

N = 8192
P = 128          # partitions / MUL0
A = 32           # attr channels
NCORES = 8
NPC = N // NCORES          # 1024 nodes per core
NCHUNK = NPC // P          # 8 chunks per core
NBLK = 8                   # node blocks per core
BLK = NPC // NBLK          # 128 nodes
GS = 4                     # s-side tiles per block   (u = 32*tu + p%32)
GA = 8                     # attr-side tiles per block (v = 4*tv + p//32)
KT = GS * GA               # 32 k-tiles
HGA = GA // 2              # 4 attr tiles per half
SCOLS = GS * BLK           # 1024 s cols per block
ACOLS = HGA * BLK          # 1024 attr cols per half
PIECE = SCOLS + 2 * ACOLS  # 3072
NGRP = 8                   # stage-3/4 groups
GRP = NPC // NGRP          # 256 nodes per group
# per-half mult routing: 16 halves (2 per block)
MULT_ROUTE = ["pool", "dve2x"] * 6 + ["pool", "dve"] + ["dve", "dve"]
WARMUP_MM = 66

F32 = mybir.dt.float32
BF16 = mybir.dt.bfloat16
BF = ml_dtypes.bfloat16

# wconst (bf16) column offsets
OFF_W1 = 0
OFF_W2 = OFF_W1 + KT * P          # 4096
OFF_SEL = OFF_W2 + GA * P         # 5120
OFF_W3 = OFF_SEL + A              # w3n on partitions 0..31
OFF_W4 = OFF_W3 + A               # w4n on partitions 0..31
FWC = OFF_W4 + 1                  # 5185
FCC = 3                           # cconst (f32): b1col | b3col | b4col


_CACHE = {}
LAST_RESULT = None         # test harness reads exec_time_ns from here


def _build():
    nc = bacc.Bacc(trn_type="TRN2", target_bir_lowering=False, debug=False)

    wconst_d = nc.dram_tensor("wconst", [P, FWC], BF16, kind="ExternalInput")
    cconst_d = nc.dram_tensor("cconst", [P, FCC], F32, kind="ExternalInput")
    sbc_d = nc.dram_tensor("sbc", [P, NBLK * PIECE], BF16,
                           kind="ExternalInput")
    out_d = nc.dram_tensor("out", [1, NPC], F32, kind="ExternalOutput")

    Alu = mybir.AluOpType
    Act = mybir.ActivationFunctionType

    with TileContext(nc) as tc:
        with (
            tc.tile_pool(name="const", bufs=1) as cp,
            tc.tile_pool(name="pc", bufs=6) as pc_p,
            tc.tile_pool(name="za", bufs=2) as za_p,
            tc.tile_pool(name="zb", bufs=2) as zb_p,
            tc.tile_pool(name="sact", bufs=3) as sact_p,
            tc.tile_pool(name="t2c", bufs=3) as t2c_p,
            tc.tile_pool(name="p2", bufs=8) as p2_p,
            tc.tile_pool(name="h2t", bufs=2) as h2t_p,
            tc.tile_pool(name="h3t", bufs=2) as h3t_p,
            tc.tile_pool(name="psacc", bufs=2, space="PSUM") as ps_acc,
            tc.tile_pool(name="pst2", bufs=4, space="PSUM") as ps_t2,
            tc.tile_pool(name="psh2", bufs=2, space="PSUM") as ps_h2,
        ):
            wconst = cp.tile([P, FWC], BF16, tag="wconst")
            cconst = cp.tile([P, FCC], F32, tag="cconst")
            pcs = {}

            def dma_sbc(q, split=False):
                pc = pc_p.tile([P, PIECE], BF16, tag="pc", name=f"pc{q}")
                if split:
                    nc.sync.dma_start(
                        pc[:, 0:SCOLS + ACOLS],
                        sbc_d.ap()[:, q * PIECE:q * PIECE + SCOLS + ACOLS])
                    nc.sync.dma_start(
                        pc[:, SCOLS + ACOLS:],
                        sbc_d.ap()[:, q * PIECE + SCOLS + ACOLS:
                                   (q + 1) * PIECE])
                else:
                    nc.sync.dma_start(
                        pc[:], sbc_d.ap()[:, q * PIECE:(q + 1) * PIECE])
                pcs[q] = pc

            # DMA dispatch order: SP ring carries sbc pieces; ACT ring
            # carries w1 (quarters, so S1(0) unblocks progressively), the
            # rest of wconst, and cconst.  Dispatch interleaves so the
            # shared wire serves the z-critical pieces first.
            WQ = KT * P // 4
            dma_sbc(0, split=True)
            nc.scalar.dma_start(wconst[:, 0:WQ], wconst_d.ap()[:, 0:WQ])
            dma_sbc(1, split=True)
            nc.scalar.dma_start(wconst[:, WQ:2 * WQ],
                                wconst_d.ap()[:, WQ:2 * WQ])
            dma_sbc(2, split=True)
            nc.scalar.dma_start(wconst[:, 2 * WQ:3 * WQ],
                                wconst_d.ap()[:, 2 * WQ:3 * WQ])
            dma_sbc(3, split=True)
            nc.scalar.dma_start(wconst[:, 3 * WQ:4 * WQ],
                                wconst_d.ap()[:, 3 * WQ:4 * WQ])
            nc.scalar.dma_start(wconst[:, 4 * WQ:], wconst_d.ap()[:, 4 * WQ:])
            nc.scalar.dma_start(cconst[:], cconst_d.ap())
            for q in range(4, NBLK):
                dma_sbc(q, split=True)

            # ---- warm-up: hoist all activation-table loads to t~0 and
            # bridge the PE through the DMA fill with dummy matmuls so the
            # first real matmul dispatches into a ramped p-state.
            scr = cp.tile([P, 192], BF16, tag="scr")
            nc.gpsimd.memset(scr[:], 0.0)
            scrf = cp.tile([P, 3], F32, tag="scrf")
            nc.gpsimd.memset(scrf[:], 0.0)
            nc.scalar.activation(scrf[:, 1:2], scrf[:, 0:1], Act.Silu)
            nc.scalar.copy(scrf[:, 1:2], scrf[:, 0:1])
            nc.scalar.activation(scrf[:, 2:3], scrf[:, 0:1], Act.Identity,
                                 bias=0.0)
            pscr = ps_h2.tile([P, 64], F32, tag="h2ps", name="pscr")
            for _ in range(WARMUP_MM):
                nc.tensor.matmul(pscr[:], scr[:, 0:128], scr[:, 128:192],
                                 start=True, stop=True)

            def stile(q):
                return pcs[q][:, 0:SCOLS]

            def atiles(q):
                return (pcs[q][:, SCOLS:SCOLS + ACOLS],
                        pcs[q][:, SCOLS + ACOLS:])

            w1f = wconst[:, OFF_W1:OFF_W1 + KT * P]
            w2f = wconst[:, OFF_W2:OFF_W2 + GA * P]
            sel = wconst[:, OFF_SEL:OFF_SEL + A]
            w3n = wconst[0:A, OFF_W3:OFF_W3 + A]
            w4n = wconst[0:A, OFF_W4:OFF_W4 + 1]
            b1col = cconst[:, 0:1]
            b3col = cconst[0:A, 1:2]
            b4one = cconst[0:1, 2:3]

            outsb = cp.tile([1, NPC], F32, tag="outsb")

            zas, zbs, sacts, p2s, h2ps = {}, {}, {}, {}, {}

            def z_form(q, split0=False):
                """DVE: tv 0..3 (za) and tv 4..6 (zb); Pool: tv 7."""
                st, (aa, ab) = stile(q), atiles(q)
                s_v = st.rearrange("p (tu n) -> p tu n", n=BLK)
                a_va = aa.rearrange("p (tv n) -> p tv n", n=BLK)
                a_vb = ab.rearrange("p (tv n) -> p tv n", n=BLK)
                za = za_p.tile([P, 16 * BLK], BF16, tag="za", name=f"za{q}")
                zb = zb_p.tile([P, 16 * BLK], BF16, tag="zb", name=f"zb{q}")
                za_v = za[:].rearrange("p (tv tu n) -> p tv tu n",
                                       tu=GS, n=BLK)
                if split0:
                    for h in range(2):
                        nc.vector.tensor_tensor(
                            za_v[:, 2 * h:2 * h + 2],
                            s_v.unsqueeze(1).broadcast_to([P, 2, GS, BLK]),
                            a_va[:, 2 * h:2 * h + 2].unsqueeze(2)
                            .broadcast_to([P, 2, GS, BLK]),
                            Alu.mult)
                else:
                    nc.vector.tensor_tensor(
                        za_v,
                        s_v.unsqueeze(1).broadcast_to([P, 4, GS, BLK]),
                        a_va.unsqueeze(2).broadcast_to([P, 4, GS, BLK]),
                        Alu.mult)
                nc.vector.tensor_tensor(
                    zb[:, 0:12 * BLK].rearrange(
                        "p (tv tu n) -> p tv tu n", tu=GS, n=BLK),
                    s_v.unsqueeze(1).broadcast_to([P, 3, GS, BLK]),
                    a_vb[:, 0:3, :].unsqueeze(2).broadcast_to(
                        [P, 3, GS, BLK]),
                    Alu.mult)
                nc.gpsimd.tensor_tensor(
                    zb[:, 12 * BLK:].rearrange(
                        "p (tu n) -> p tu n", n=BLK),
                    s_v,
                    a_vb[:, 3:4, :].broadcast_to([P, GS, BLK]),
                    Alu.mult)
                zas[q], zbs[q] = za, zb

            def s1_matmuls(q):
                acc = ps_acc.tile([P, BLK], F32, tag="acc", name=f"acc{q}")
                for t in range(KT):
                    z = zas[q] if t < KT // 2 else zbs[q]
                    zc = (t % (KT // 2)) * BLK
                    nc.tensor.matmul(
                        acc[:], w1f[:, t * P:(t + 1) * P],
                        z[:, zc:zc + BLK],
                        start=(t == 0), stop=(t == KT - 1))
                sact = sact_p.tile([P, BLK], BF16, tag="sact",
                                   name=f"sact{q}")
                nc.scalar.activation(sact[:], acc[:], Act.Silu, bias=b1col)
                sacts[q] = sact

            def t2t_and_mult(q):
                """Per half: 4 T2T matmuls, then the attr multiply.
                Route 'pool': ACT copy PSUM->SBUF bf16, Pool multiplies.
                Route 'dve2x': ACT copy, DVE multiplies at the 2x rate.
                Route 'dve': DVE multiplies straight from PSUM (no copy —
                used only at the drain to shorten the last chain)."""
                aa, ab = atiles(q)
                outs = []
                for half, asrc in enumerate((aa, ab)):
                    route = MULT_ROUTE[2 * q + half]
                    t2 = ps_t2.tile([P, ACOLS], F32, tag="t2",
                                    name=f"t2_{q}_{half}")
                    for bb in range(HGA):
                        nc.tensor.matmul(
                            t2[:, bb * BLK:(bb + 1) * BLK],
                            w2f[:, (half * HGA + bb) * P:
                                (half * HGA + bb + 1) * P],
                            sacts[q][:], start=True, stop=True)
                    p2 = p2_p.tile([P, ACOLS], BF16, tag="p2",
                                   name=f"p2_{q}_{half}")
                    if route == "dve":
                        nc.vector.tensor_tensor(p2[:], t2[:], asrc,
                                                Alu.mult)
                    else:
                        t2c = t2c_p.tile([P, ACOLS], BF16, tag="t2c",
                                         name=f"t2c_{q}_{half}")
                        nc.scalar.copy(t2c[:], t2[:])
                        eng = nc.gpsimd if route == "pool" else nc.vector
                        eng.tensor_tensor(p2[:], t2c[:], asrc, Alu.mult)
                    outs.append(p2)
                p2s[q] = tuple(outs)

            def sel_matmuls(q):
                """block q (one 128-node chunk): 8 accumulating matmuls
                with lhsT = Sel -> h2ps[q] [32w, n] (transposed)."""
                h2ps[q] = ps_h2.tile([A, GRP], F32, tag="h2ps",
                                     name=f"h2ps{q}")
                p2a, p2b = p2s[q]
                dst = h2ps[q][:]
                for b in range(GA):
                    src = p2a if b < HGA else p2b
                    c0 = (b % HGA) * BLK
                    nc.tensor.matmul(dst, sel, src[:, c0:c0 + BLK],
                                     start=(b == 0), stop=(b == GA - 1))

            def s34(g):
                """stages 3/4 for the 256-node group g, [feature, node]."""
                h2t = h2t_p.tile([A, GRP], BF16, tag="h2t", name=f"h2t{g}")
                nc.scalar.copy(h2t[:], h2ps[g][:])
                o3 = ps_acc.tile([A, GRP], F32, tag="acc", name=f"o3{g}")
                nc.tensor.matmul(o3[:], w3n, h2t[:], start=True, stop=True)
                h3t = h3t_p.tile([A, GRP], BF16, tag="h3t", name=f"h3t{g}")
                nc.scalar.activation(h3t[:], o3[:], Act.Silu, bias=b3col)
                o4 = ps_acc.tile([1, GRP], F32, tag="acc", name=f"o4{g}")
                nc.tensor.matmul(o4[:], w4n, h3t[:], start=True, stop=True)
                nc.scalar.activation(outsb[:, g * GRP:(g + 1) * GRP], o4[:],
                                     Act.Identity, bias=b4one)
                nc.sync.dma_start(out_d.ap()[:, g * GRP:(g + 1) * GRP],
                                  outsb[:, g * GRP:(g + 1) * GRP])

            # ---- pipelined emission ----
            z_form(0, split0=True)
            z_form(1)
            for i in range(NBLK):
                if i + 2 < NBLK:
                    z_form(i + 2)
                s1_matmuls(i)
                if i >= 1:
                    t2t_and_mult(i - 1)
                if i == NBLK - 1:
                    t2t_and_mult(i)
                if i >= 3:
                    sel_matmuls(i - 3)
                    s34(i - 3)
            # tail
            sel_matmuls(NBLK - 3)
            s34(NBLK - 3)
            sel_matmuls(NBLK - 2)
            s34(NBLK - 2)
            sel_matmuls(NBLK - 1)
            s34(NBLK - 1)

    nc.compile()
    return nc


def _get_nc():
    if "nc" not in _CACHE:
        _CACHE["nc"] = _build()
    return _CACHE["nc"]


def _prep_inputs(node_vec, node_embedding, W1s, b1s, W2, b2, W3, b3, W4, b4):
    f = np.float32
    inv = f(1.0 / 64.0)                      # 1/sqrt(128*32)
    s = np.ascontiguousarray(node_vec[:, :P]).astype(f)
    attr = np.asarray(node_embedding, f)

    pidx = np.arange(P)
    # k-tile tau = 4*tv + tu:  u = 32*tu + p%32,  v = 4*tv + p//32
    su_rows = A * np.arange(GS)[:, None] + (pidx % A)[None, :]    # [GS, P]
    av_rows = GS * np.arange(GA)[:, None] + (pidx // A)[None, :]  # [GA, P]

    wconst = np.zeros((P, FWC), BF)
    w1 = (np.asarray(W1s, f) * inv).astype(BF)           # [128u, 32v, 128w]
    for tv in range(GA):
        for tu in range(GS):
            t = GS * tv + tu
            wconst[:, OFF_W1 + t * P:OFF_W1 + (t + 1) * P] = \
                w1[su_rows[tu], av_rows[tv], :]
    # W2f_b[u, p2] = W2[u, 4b + p2//32, p2%32] * inv
    w2 = np.asarray(W2, f) * inv                         # [128u, 32v, 32w]
    for b in range(GA):
        wconst[:, OFF_W2 + b * P:OFF_W2 + (b + 1) * P] = \
            w2[:, GS * b + pidx // A, pidx % A].astype(BF)
    wconst[:, OFF_SEL:OFF_SEL + A] = \
        (pidx[:, None] % A == np.arange(A)[None, :]).astype(BF)
    w3n = np.asarray(W3, f) / np.sqrt(f(A))
    w4n = np.asarray(W4, f) / np.sqrt(f(A))
    wconst[0:A, OFF_W3:OFF_W3 + A] = w3n.astype(BF)
    wconst[0:A, OFF_W4:OFF_W4 + 1] = w4n.astype(BF)

    b3_eff = np.asarray(b3, f) + np.asarray(b2, f) @ w3n
    cconst = np.zeros((P, FCC), f)
    cconst[:, 0] = np.asarray(b1s, f)
    cconst[0:A, 1] = b3_eff
    cconst[0, 2] = np.asarray(b4, f).reshape(-1)[0]

    in_maps = []
    for core in range(NCORES):
        lo = core * NPC
        S = s[lo:lo + NPC].astype(BF)                     # [1024, 128]
        atb = attr[lo:lo + NPC].astype(BF)                # [1024, 32]

        sbc = np.empty((P, NBLK * PIECE), BF)
        for q in range(NBLK):
            Sb = S[q * BLK:(q + 1) * BLK]                 # [BLK, 128]
            Ab = atb[q * BLK:(q + 1) * BLK]               # [BLK, 32]
            base = q * PIECE
            for g in range(GS):
                sbc[:, base + g * BLK: base + (g + 1) * BLK] = Sb.T[su_rows[g]]
            base += GS * BLK
            for g in range(GA):
                sbc[:, base + g * BLK: base + (g + 1) * BLK] = Ab.T[av_rows[g]]

        in_maps.append(dict(wconst=wconst, cconst=cconst, sbc=sbc))
    return in_maps


def kernel(**inputs):
    global LAST_RESULT
    trace = bool(int(os.environ.get("KERNEL_TRACE", "0")))
    in_maps = _prep_inputs(
        inputs["node_vec"], inputs["node_embedding"],
        inputs["W1s"], inputs["b1s"], inputs["W2"], inputs["b2"],
        inputs["W3"], inputs["b3"], inputs["W4"], inputs["b4"],
    )
    nc = _get_nc()
    res = bass_utils.run_bass_kernel_spmd(
        nc, in_maps, core_ids=list(range(NCORES)), trace=trace)
    LAST_RESULT = res
    outs = [np.asarray(res.results[i]["out"]) for i in range(NCORES)]
    energy = np.concatenate([o.reshape(NPC) for o in outs]).reshape(N, 1)
    return energy.astype(np.float32)


# revision 5
# speedup vs baseline: 1.0807x; 1.0759x over previous
"""Trainium2 Bass kernel for nn_EquivariantScalar_viaTP — V3.3.

Reference computation (after dead-code elimination — the gate / l=1 / l=2
paths never reach the output):

    s      = node_vec[:, :128]                                  # [N, 128]
    attr   = node_embedding                                     # [N, 32]
    s_mid  = einsum('nu,nv,uvw->nw', s, attr, W1s) / 64 + b1s   # [N, 128]
    s_act  = silu(s_mid)
    h      = einsum('nu,nv,uvw->nw', s_act, attr, W2) / 64 + b2 # [N, 32]
    h      = silu(h @ (W3/sqrt(32)) + b3)                       # [N, 32]
    out    = h @ (W4/sqrt(32)) + b4                             # [N, 1]

Sharding: node dim N=8192 across 8 cores (1024 nodes each).

V3 design (engine-balanced, cost-model driven):
  Stage 1 — Z-outer-product form: s_mid^T[w,n] = sum_k W1f[k,w] Z[k,n],
  k=(u,v), 32 accumulating bf16 k-tile matmuls per 256-node block.
  k-tile tau=(tv,tu): partition p maps u = 32*tu + p%32, v = 4*tv + p//32.
  Z is formed elementwise from replicated sT / attrT tiles; broadcasts on
  the two OUTER free dims keep the 2x bf16 DVE mode.  The DVE forms
  tv 0..6 (two fused ops), the otherwise-idle Pool engine forms tv 7.
  silu(+b1) -> sact^T [u,n] bf16.

  Stage 2 transposed (T2T): per half-block, 4 matmuls with lhsT = W2f_b
  [128u, 128(v,w)] and rhs = sact^T produce T2T[(v,w), n] in PSUM
  (v(p2)=4b+p2//32, w(p2)=p2%32).  The attr multiply in2 is the SAME sbc
  attr tile stage 1 streams (its partition map matches v(p2) by
  construction).  GPSIMD cannot read PSUM (walrus rejects it), so blocks
  0..1 route ACT-copy(PSUM->SBUF bf16) -> Pool multiply, and blocks 2..3
  (DVE idle by then) multiply on the DVE straight from PSUM.
  The v-contraction is 8 accumulating PE matmuls per 128-node chunk with
  lhsT = a 0/1 selector Sel[p2,w] = (p2%32==w) and rhs = P2 — h2 lands
  TRANSPOSED [32w, n] in PSUM, so stages 3/4 need no PE transposes at
  all.  b2 folds into b3 (b3_eff = b3 + b2 @ W3/sqrt(A)).

  Stages 3/4 per 512-node group, all in the [feature, node] orientation:
  one ACT copy (PSUM->SBUF bf16), matmul lhsT=W3n, silu with per-
  partition bias b3, matmul lhsT=W4n, biased Identity copy -> out row
  [1, 512].  Output is [1, 1024] per core, node-ordered.

  The PE stream is emitted interleaved (S1(i) | T2T(i-1) | Sel(i-2)) and
  a tunable dummy-matmul warm-up bridges the PE through the DMA fill so
  real matmuls dispatch into a fully ramped p-state (the cost model
  charges p-state at dispatch; idle resets the ramp).
"""

import os

import numpy as np
import ml_dtypes

import concourse.bass as bass
import concourse.bacc as bacc
import concourse.mybir as mybir
from concourse.tile import TileContext
from concourse import bass_utils

N = 8192
P = 128          # partitions / MUL0
A = 32           # attr channels
NCORES = 8
NPC = N // NCORES          # 1024 nodes per core
NCHUNK = NPC // P          # 8 chunks per core
NBLK = 4                   # node blocks per core
BLK = NPC // NBLK          # 128 nodes
GS = 4                     # s-side tiles per block   (u = 32*tu + p%32)
GA = 8                     # attr-side tiles per block (v = 4*tv + p//32)
KT = GS * GA               # 32 k-tiles
HGA = GA // 2              # 4 attr tiles per half
SCOLS = GS * BLK           # 1024 s cols per block
ACOLS = HGA * BLK          # 1024 attr cols per half
PIECE = SCOLS + 2 * ACOLS  # 3072
NGRP = 4                   # stage-3/4 groups
GRP = NPC // NGRP          # 256 nodes per group
# per-half mult routing: 16 halves (2 per block)
MULT_ROUTE = ["pool", "dve2x"] * 2 + ["dve2x", "dve"] + ["dve", "dve"]
WARMUP_MM = 66

F32 = mybir.dt.float32
BF16 = mybir.dt.bfloat16
BF = ml_dtypes.bfloat16

# wconst (bf16) column offsets
OFF_W1 = 0
OFF_W2 = OFF_W1 + KT * P          # 4096
OFF_SEL = OFF_W2 + GA * P         # 5120
OFF_W3 = OFF_SEL + A              # w3n on partitions 0..31
OFF_W4 = OFF_W3 + A               # w4n on partitions 0..31
FWC = OFF_W4 + 1                  # 5185
FCC = 3                           # cconst (f32): b1col | b3col | b4col


_CACHE = {}
LAST_RESULT = None         # test harness reads exec_time_ns from here


def _build():
    nc = bacc.Bacc(trn_type="TRN2", target_bir_lowering=False, debug=False)

    wconst_d = nc.dram_tensor("wconst", [P, FWC], BF16, kind="ExternalInput")
    cconst_d = nc.dram_tensor("cconst", [P, FCC], F32, kind="ExternalInput")
    sbc_d = nc.dram_tensor("sbc", [P, NBLK * PIECE], BF16,
                           kind="ExternalInput")
    out_d = nc.dram_tensor("out", [1, NPC], F32, kind="ExternalOutput")

    Alu = mybir.AluOpType
    Act = mybir.ActivationFunctionType

    with TileContext(nc) as tc:
        with (
            tc.tile_pool(name="const", bufs=1) as cp,
            tc.tile_pool(name="pc", bufs=8) as pc_p,
            tc.tile_pool(name="za", bufs=2) as za_p,
            tc.tile_pool(name="zb", bufs=2) as zb_p,
            tc.tile_pool(name="sact", bufs=3) as sact_p,
            tc.tile_pool(name="t2c", bufs=3) as t2c_p,
            tc.tile_pool(name="p2", bufs=8) as p2_p,
            tc.tile_pool(name="h2t", bufs=2) as h2t_p,
            tc.tile_pool(name="h3t", bufs=2) as h3t_p,
            tc.tile_pool(name="psacc", bufs=2, space="PSUM") as ps_acc,
            tc.tile_pool(name="pst2", bufs=2, space="PSUM") as ps_t2,
            tc.tile_pool(name="psh2", bufs=2, space="PSUM") as ps_h2,
        ):
            wconst = cp.tile([P, FWC], BF16, tag="wconst")
            cconst = cp.tile([P, FCC], F32, tag="cconst")
            pcs = {}

            def dma_sbc(q, split=False):
                pc = pc_p.tile([P, PIECE], BF16, tag="pc", name=f"pc{q}")
                if split:
                    nc.sync.dma_start(
                        pc[:, 0:SCOLS + ACOLS],
                        sbc_d.ap()[:, q * PIECE:q * PIECE + SCOLS + ACOLS])
                    nc.sync.dma_start(
                        pc[:, SCOLS + ACOLS:],
                        sbc_d.ap()[:, q * PIECE + SCOLS + ACOLS:
                                   (q + 1) * PIECE])
                else:
                    nc.sync.dma_start(
                        pc[:], sbc_d.ap()[:, q * PIECE:(q + 1) * PIECE])
                pcs[q] = pc

            # DMA dispatch order: SP ring carries sbc pieces; ACT ring
            # carries w1 (quarters, so S1(0) unblocks progressively), the
            # rest of wconst, and cconst.  Dispatch interleaves so the
            # shared wire serves the z-critical pieces first.
            WQ = KT * P // 4
            dma_sbc(0, split=True)
            nc.scalar.dma_start(wconst[:, 0:WQ], wconst_d.ap()[:, 0:WQ])
            dma_sbc(1, split=True)
            nc.scalar.dma_start(wconst[:, WQ:2 * WQ],
                                wconst_d.ap()[:, WQ:2 * WQ])
            dma_sbc(2, split=True)
            nc.scalar.dma_start(wconst[:, 2 * WQ:3 * WQ],
                                wconst_d.ap()[:, 2 * WQ:3 * WQ])
            dma_sbc(3, split=True)
            nc.scalar.dma_start(wconst[:, 3 * WQ:4 * WQ],
                                wconst_d.ap()[:, 3 * WQ:4 * WQ])
            nc.scalar.dma_start(wconst[:, 4 * WQ:], wconst_d.ap()[:, 4 * WQ:])
            nc.scalar.dma_start(cconst[:], cconst_d.ap())
            for q in range(4, NBLK):
                dma_sbc(q, split=True)

            # ---- warm-up: hoist all activation-table loads to t~0 and
            # bridge the PE through the DMA fill with dummy matmuls so the
            # first real matmul dispatches into a ramped p-state.
            scr = cp.tile([P, 192], BF16, tag="scr")
            nc.gpsimd.memset(scr[:], 0.0)
            scrf = cp.tile([P, 3], F32, tag="scrf")
            nc.gpsimd.memset(scrf[:], 0.0)
            nc.scalar.activation(scrf[:, 1:2], scrf[:, 0:1], Act.Silu)
            nc.scalar.copy(scrf[:, 1:2], scrf[:, 0:1])
            nc.scalar.activation(scrf[:, 2:3], scrf[:, 0:1], Act.Identity,
                                 bias=0.0)
            pscr = ps_h2.tile([P, 64], F32, tag="h2ps", name="pscr")
            for _ in range(WARMUP_MM):
                nc.tensor.matmul(pscr[:], scr[:, 0:128], scr[:, 128:192],
                                 start=True, stop=True)

            def stile(q):
                return pcs[q][:, 0:SCOLS]

            def atiles(q):
                return (pcs[q][:, SCOLS:SCOLS + ACOLS],
                        pcs[q][:, SCOLS + ACOLS:])

            w1f = wconst[:, OFF_W1:OFF_W1 + KT * P]
            w2f = wconst[:, OFF_W2:OFF_W2 + GA * P]
            sel = wconst[:, OFF_SEL:OFF_SEL + A]
            w3n = wconst[0:A, OFF_W3:OFF_W3 + A]
            w4n = wconst[0:A, OFF_W4:OFF_W4 + 1]
            b1col = cconst[:, 0:1]
            b3col = cconst[0:A, 1:2]
            b4one = cconst[0:1, 2:3]

            outsb = cp.tile([1, NPC], F32, tag="outsb")

            zas, zbs, sacts, p2s, h2ps = {}, {}, {}, {}, {}

            def z_form(q, split0=False):
                """DVE: tv 0..3 (za) and tv 4..6 (zb); Pool: tv 7."""
                st, (aa, ab) = stile(q), atiles(q)
                s_v = st.rearrange("p (tu n) -> p tu n", n=BLK)
                a_va = aa.rearrange("p (tv n) -> p tv n", n=BLK)
                a_vb = ab.rearrange("p (tv n) -> p tv n", n=BLK)
                za = za_p.tile([P, 16 * BLK], BF16, tag="za", name=f"za{q}")
                zb = zb_p.tile([P, 16 * BLK], BF16, tag="zb", name=f"zb{q}")
                za_v = za[:].rearrange("p (tv tu n) -> p tv tu n",
                                       tu=GS, n=BLK)
                if split0:
                    for h in range(2):
                        nc.vector.tensor_tensor(
                            za_v[:, 2 * h:2 * h + 2],
                            s_v.unsqueeze(1).broadcast_to([P, 2, GS, BLK]),
                            a_va[:, 2 * h:2 * h + 2].unsqueeze(2)
                            .broadcast_to([P, 2, GS, BLK]),
                            Alu.mult)
                else:
                    nc.vector.tensor_tensor(
                        za_v,
                        s_v.unsqueeze(1).broadcast_to([P, 4, GS, BLK]),
                        a_va.unsqueeze(2).broadcast_to([P, 4, GS, BLK]),
                        Alu.mult)
                nc.vector.tensor_tensor(
                    zb[:, 0:12 * BLK].rearrange(
                        "p (tv tu n) -> p tv tu n", tu=GS, n=BLK),
                    s_v.unsqueeze(1).broadcast_to([P, 3, GS, BLK]),
                    a_vb[:, 0:3, :].unsqueeze(2).broadcast_to(
                        [P, 3, GS, BLK]),
                    Alu.mult)
                nc.gpsimd.tensor_tensor(
                    zb[:, 12 * BLK:].rearrange(
                        "p (tu n) -> p tu n", n=BLK),
                    s_v,
                    a_vb[:, 3:4, :].broadcast_to([P, GS, BLK]),
                    Alu.mult)
                zas[q], zbs[q] = za, zb

            def s1_matmuls(q):
                acc = ps_acc.tile([P, BLK], F32, tag="acc", name=f"acc{q}")
                for t in range(KT):
                    z = zas[q] if t < KT // 2 else zbs[q]
                    zc = (t % (KT // 2)) * BLK
                    nc.tensor.matmul(
                        acc[:], w1f[:, t * P:(t + 1) * P],
                        z[:, zc:zc + BLK],
                        start=(t == 0), stop=(t == KT - 1))
                sact = sact_p.tile([P, BLK], BF16, tag="sact",
                                   name=f"sact{q}")
                nc.scalar.activation(sact[:], acc[:], Act.Silu, bias=b1col)
                sacts[q] = sact

            def t2t_and_mult(q):
                """Per half: 4 T2T matmuls, then the attr multiply.
                Route 'pool': ACT copy PSUM->SBUF bf16, Pool multiplies.
                Route 'dve2x': ACT copy, DVE multiplies at the 2x rate.
                Route 'dve': DVE multiplies straight from PSUM (no copy —
                used only at the drain to shorten the last chain)."""
                aa, ab = atiles(q)
                outs = []
                for half, asrc in enumerate((aa, ab)):
                    route = MULT_ROUTE[2 * q + half]
                    t2 = ps_t2.tile([P, ACOLS], F32, tag="t2",
                                    name=f"t2_{q}_{half}")
                    for bb in range(HGA):
                        nc.tensor.matmul(
                            t2[:, bb * BLK:(bb + 1) * BLK],
                            w2f[:, (half * HGA + bb) * P:
                                (half * HGA + bb + 1) * P],
                            sacts[q][:], start=True, stop=True)
                    p2 = p2_p.tile([P, ACOLS], BF16, tag="p2",
                                   name=f"p2_{q}_{half}")
                    if route == "dve":
                        nc.vector.tensor_tensor(p2[:], t2[:], asrc,
                                                Alu.mult)
                    else:
                        t2c = t2c_p.tile([P, ACOLS], BF16, tag="t2c",
                                         name=f"t2c_{q}_{half}")
                        nc.scalar.copy(t2c[:], t2[:])
                        eng = nc.gpsimd if route == "pool" else nc.vector
                        eng.tensor_tensor(p2[:], t2c[:], asrc, Alu.mult)
                    outs.append(p2)
                p2s[q] = tuple(outs)

            def sel_matmuls(q):
                """block q (one 128-node chunk): 8 accumulating matmuls
                with lhsT = Sel -> h2ps[q] [32w, n] (transposed)."""
                h2ps[q] = ps_h2.tile([A, GRP], F32, tag="h2ps",
                                     name=f"h2ps{q}")
                p2a, p2b = p2s[q]
                dst = h2ps[q][:]
                for b in range(GA):
                    src = p2a if b < HGA else p2b
                    c0 = (b % HGA) * BLK
                    nc.tensor.matmul(dst, sel, src[:, c0:c0 + BLK],
                                     start=(b == 0), stop=(b == GA - 1))

            def s34(g):
                """stages 3/4 for the 256-node group g, [feature, node]."""
                h2t = h2t_p.tile([A, GRP], BF16, tag="h2t", name=f"h2t{g}")
                nc.scalar.copy(h2t[:], h2ps[g][:])
                o3 = ps_acc.tile([A, GRP], F32, tag="acc", name=f"o3{g}")
                nc.tensor.matmul(o3[:], w3n, h2t[:], start=True, stop=True)
                h3t = h3t_p.tile([A, GRP], BF16, tag="h3t", name=f"h3t{g}")
                nc.scalar.activation(h3t[:], o3[:], Act.Silu, bias=b3col)
                o4 = ps_acc.tile([1, GRP], F32, tag="acc", name=f"o4{g}")
                nc.tensor.matmul(o4[:], w4n, h3t[:], start=True, stop=True)
                nc.scalar.activation(outsb[:, g * GRP:(g + 1) * GRP], o4[:],
                                     Act.Identity, bias=b4one)
                nc.sync.dma_start(out_d.ap()[:, g * GRP:(g + 1) * GRP],
                                  outsb[:, g * GRP:(g + 1) * GRP])

            # ---- pipelined emission ----
            z_form(0, split0=True)
            z_form(1)
            for i in range(NBLK):
                if i + 2 < NBLK:
                    z_form(i + 2)
                s1_matmuls(i)
                if i >= 1:
                    t2t_and_mult(i - 1)
                if i == NBLK - 1:
                    t2t_and_mult(i)
                if i >= 3:
                    sel_matmuls(i - 3)
                    s34(i - 3)
            # tail
            sel_matmuls(NBLK - 3)
            s34(NBLK - 3)
            sel_matmuls(NBLK - 2)
            s34(NBLK - 2)
            sel_matmuls(NBLK - 1)
            s34(NBLK - 1)

    nc.compile()
    return nc


def _get_nc():
    if "nc" not in _CACHE:
        _CACHE["nc"] = _build()
    return _CACHE["nc"]


def _prep_inputs(node_vec, node_embedding, W1s, b1s, W2, b2, W3, b3, W4, b4):
    f = np.float32
    inv = f(1.0 / 64.0)                      # 1/sqrt(128*32)
    s = np.ascontiguousarray(node_vec[:, :P]).astype(f)
    attr = np.asarray(node_embedding, f)

    pidx = np.arange(P)
    # k-tile tau = 4*tv + tu:  u = 32*tu + p%32,  v = 4*tv + p//32
    su_rows = A * np.arange(GS)[:, None] + (pidx % A)[None, :]    # [GS, P]
    av_rows = GS * np.arange(GA)[:, None] + (pidx // A)[None, :]  # [GA, P]

    wconst = np.zeros((P, FWC), BF)
    w1 = (np.asarray(W1s, f) * inv).astype(BF)           # [128u, 32v, 128w]
    for tv in range(GA):
        for tu in range(GS):
            t = GS * tv + tu
            wconst[:, OFF_W1 + t * P:OFF_W1 + (t + 1) * P] = \
                w1[su_rows[tu], av_rows[tv], :]
    # W2f_b[u, p2] = W2[u, 4b + p2//32, p2%32] * inv
    w2 = np.asarray(W2, f) * inv                         # [128u, 32v, 32w]
    for b in range(GA):
        wconst[:, OFF_W2 + b * P:OFF_W2 + (b + 1) * P] = \
            w2[:, GS * b + pidx // A, pidx % A].astype(BF)
    wconst[:, OFF_SEL:OFF_SEL + A] = \
        (pidx[:, None] % A == np.arange(A)[None, :]).astype(BF)
    w3n = np.asarray(W3, f) / np.sqrt(f(A))
    w4n = np.asarray(W4, f) / np.sqrt(f(A))
    wconst[0:A, OFF_W3:OFF_W3 + A] = w3n.astype(BF)
    wconst[0:A, OFF_W4:OFF_W4 + 1] = w4n.astype(BF)

    b3_eff = np.asarray(b3, f) + np.asarray(b2, f) @ w3n
    cconst = np.zeros((P, FCC), f)
    cconst[:, 0] = np.asarray(b1s, f)
    cconst[0:A, 1] = b3_eff
    cconst[0, 2] = np.asarray(b4, f).reshape(-1)[0]

    in_maps = []
    for core in range(NCORES):
        lo = core * NPC
        S = s[lo:lo + NPC].astype(BF)                     # [1024, 128]
        atb = attr[lo:lo + NPC].astype(BF)                # [1024, 32]

        sbc = np.empty((P, NBLK * PIECE), BF)
        for q in range(NBLK):
            Sb = S[q * BLK:(q + 1) * BLK]                 # [BLK, 128]
            Ab = atb[q * BLK:(q + 1) * BLK]               # [BLK, 32]
            base = q * PIECE
            for g in range(GS):
                sbc[:, base + g * BLK: base + (g + 1) * BLK] = Sb.T[su_rows[g]]
            base += GS * BLK
            for g in range(GA):
                sbc[:, base + g * BLK: base + (g + 1) * BLK] = Ab.T[av_rows[g]]

        in_maps.append(dict(wconst=wconst, cconst=cconst, sbc=sbc))
    return in_maps


def kernel(**inputs):
    global LAST_RESULT
    trace = bool(int(os.environ.get("KERNEL_TRACE", "0")))
    in_maps = _prep_inputs(
        inputs["node_vec"], inputs["node_embedding"],
        inputs["W1s"], inputs["b1s"], inputs["W2"], inputs["b2"],
        inputs["W3"], inputs["b3"], inputs["W4"], inputs["b4"],
    )
    nc = _get_nc()
    res = bass_utils.run_bass_kernel_spmd(
        nc, in_maps, core_ids=list(range(NCORES)), trace=trace)
    LAST_RESULT = res
    outs = [np.asarray(res.results[i]["out"]) for i in range(NCORES)]
    energy = np.concatenate([o.reshape(NPC) for o in outs]).reshape(N, 1)
    return energy.astype(np.float32)


# revision 6
# speedup vs baseline: 1.0913x; 1.0098x over previous
"""Trainium2 Bass kernel for nn_EquivariantScalar_viaTP — V3.3.

Reference computation (after dead-code elimination — the gate / l=1 / l=2
paths never reach the output):

    s      = node_vec[:, :128]                                  # [N, 128]
    attr   = node_embedding                                     # [N, 32]
    s_mid  = einsum('nu,nv,uvw->nw', s, attr, W1s) / 64 + b1s   # [N, 128]
    s_act  = silu(s_mid)
    h      = einsum('nu,nv,uvw->nw', s_act, attr, W2) / 64 + b2 # [N, 32]
    h      = silu(h @ (W3/sqrt(32)) + b3)                       # [N, 32]
    out    = h @ (W4/sqrt(32)) + b4                             # [N, 1]

Sharding: node dim N=8192 across 8 cores (1024 nodes each).

V3 design (engine-balanced, cost-model driven):
  Stage 1 — Z-outer-product form: s_mid^T[w,n] = sum_k W1f[k,w] Z[k,n],
  k=(u,v), 32 accumulating bf16 k-tile matmuls per 256-node block.
  k-tile tau=(tv,tu): partition p maps u = 32*tu + p%32, v = 4*tv + p//32.
  Z is formed elementwise from replicated sT / attrT tiles; broadcasts on
  the two OUTER free dims keep the 2x bf16 DVE mode.  The DVE forms
  tv 0..6 (two fused ops), the otherwise-idle Pool engine forms tv 7.
  silu(+b1) -> sact^T [u,n] bf16.

  Stage 2 transposed (T2T): per half-block, 4 matmuls with lhsT = W2f_b
  [128u, 128(v,w)] and rhs = sact^T produce T2T[(v,w), n] in PSUM
  (v(p2)=4b+p2//32, w(p2)=p2%32).  The attr multiply in2 is the SAME sbc
  attr tile stage 1 streams (its partition map matches v(p2) by
  construction).  GPSIMD cannot read PSUM (walrus rejects it), so blocks
  0..1 route ACT-copy(PSUM->SBUF bf16) -> Pool multiply, and blocks 2..3
  (DVE idle by then) multiply on the DVE straight from PSUM.
  The v-contraction is 8 accumulating PE matmuls per 128-node chunk with
  lhsT = a 0/1 selector Sel[p2,w] = (p2%32==w) and rhs = P2 — h2 lands
  TRANSPOSED [32w, n] in PSUM, so stages 3/4 need no PE transposes at
  all.  b2 folds into b3 (b3_eff = b3 + b2 @ W3/sqrt(A)).

  Stages 3/4 per 512-node group, all in the [feature, node] orientation:
  one ACT copy (PSUM->SBUF bf16), matmul lhsT=W3n, silu with per-
  partition bias b3, matmul lhsT=W4n, biased Identity copy -> out row
  [1, 512].  Output is [1, 1024] per core, node-ordered.

  The PE stream is emitted interleaved (S1(i) | T2T(i-1) | Sel(i-2)) and
  a tunable dummy-matmul warm-up bridges the PE through the DMA fill so
  real matmuls dispatch into a fully ramped p-state (the cost model
  charges p-state at dispatch; idle resets the ramp).
"""

import os

import numpy as np
import ml_dtypes

import concourse.bass as bass
import concourse.bacc as bacc
import concourse.mybir as mybir
from concourse.tile import TileContext
from concourse import bass_utils

N = 8192
P = 128          # partitions / MUL0
A = 32           # attr channels
NCORES = 8
NPC = N // NCORES          # 1024 nodes per core
NCHUNK = NPC // P          # 8 chunks per core
NBLK = 4                   # node blocks per core
BLK = NPC // NBLK          # 128 nodes
GS = 4                     # s-side tiles per block   (u = 32*tu + p%32)
GA = 8                     # attr-side tiles per block (v = 4*tv + p//32)
KT = GS * GA               # 32 k-tiles
HGA = GA // 2              # 4 attr tiles per half
SCOLS = GS * BLK           # 1024 s cols per block
ACOLS = HGA * BLK          # 1024 attr cols per half
PIECE = SCOLS + 2 * ACOLS  # 3072
NGRP = 4                   # stage-3/4 groups
GRP = NPC // NGRP          # 256 nodes per group
# per-half mult routing: 16 halves (2 per block)
MULT_ROUTE = ["pool", "dve2x"] * 3 + ["dve", "dve"]
WARMUP_MM = 66

F32 = mybir.dt.float32
BF16 = mybir.dt.bfloat16
BF = ml_dtypes.bfloat16

# wconst (bf16) column offsets
OFF_W1 = 0
OFF_W2 = OFF_W1 + KT * P          # 4096
OFF_SEL = OFF_W2 + GA * P         # 5120
OFF_W3 = OFF_SEL + A              # w3n on partitions 0..31
OFF_W4 = OFF_W3 + A               # w4n on partitions 0..31
FWC = OFF_W4 + 1                  # 5185
FCC = 3                           # cconst (f32): b1col | b3col | b4col


_CACHE = {}
LAST_RESULT = None         # test harness reads exec_time_ns from here


def _build():
    nc = bacc.Bacc(trn_type="TRN2", target_bir_lowering=False, debug=False)

    wconst_d = nc.dram_tensor("wconst", [P, FWC], BF16, kind="ExternalInput")
    cconst_d = nc.dram_tensor("cconst", [P, FCC], F32, kind="ExternalInput")
    sbc_d = nc.dram_tensor("sbc", [P, NBLK * PIECE], BF16,
                           kind="ExternalInput")
    out_d = nc.dram_tensor("out", [1, NPC], F32, kind="ExternalOutput")

    Alu = mybir.AluOpType
    Act = mybir.ActivationFunctionType

    with TileContext(nc) as tc:
        with (
            tc.tile_pool(name="const", bufs=1) as cp,
            tc.tile_pool(name="pc", bufs=8) as pc_p,
            tc.tile_pool(name="za", bufs=2) as za_p,
            tc.tile_pool(name="zb", bufs=2) as zb_p,
            tc.tile_pool(name="sact", bufs=3) as sact_p,
            tc.tile_pool(name="t2c", bufs=3) as t2c_p,
            tc.tile_pool(name="p2", bufs=8) as p2_p,
            tc.tile_pool(name="h2t", bufs=2) as h2t_p,
            tc.tile_pool(name="h3t", bufs=2) as h3t_p,
            tc.tile_pool(name="psacc", bufs=2, space="PSUM") as ps_acc,
            tc.tile_pool(name="pst2", bufs=2, space="PSUM") as ps_t2,
            tc.tile_pool(name="psh2", bufs=2, space="PSUM") as ps_h2,
        ):
            wconst = cp.tile([P, FWC], BF16, tag="wconst")
            cconst = cp.tile([P, FCC], F32, tag="cconst")
            pcs = {}

            def dma_sbc(q, split=False):
                pc = pc_p.tile([P, PIECE], BF16, tag="pc", name=f"pc{q}")
                if split:
                    nc.sync.dma_start(
                        pc[:, 0:SCOLS + ACOLS],
                        sbc_d.ap()[:, q * PIECE:q * PIECE + SCOLS + ACOLS])
                    nc.sync.dma_start(
                        pc[:, SCOLS + ACOLS:],
                        sbc_d.ap()[:, q * PIECE + SCOLS + ACOLS:
                                   (q + 1) * PIECE])
                else:
                    nc.sync.dma_start(
                        pc[:], sbc_d.ap()[:, q * PIECE:(q + 1) * PIECE])
                pcs[q] = pc

            # DMA dispatch order: SP ring carries sbc pieces; ACT ring
            # carries w1 (quarters, so S1(0) unblocks progressively), the
            # rest of wconst, and cconst.  Dispatch interleaves so the
            # shared wire serves the z-critical pieces first.
            WQ = KT * P // 4
            dma_sbc(0, split=True)
            nc.scalar.dma_start(wconst[:, 0:WQ], wconst_d.ap()[:, 0:WQ])
            dma_sbc(1, split=True)
            nc.scalar.dma_start(wconst[:, WQ:2 * WQ],
                                wconst_d.ap()[:, WQ:2 * WQ])
            dma_sbc(2, split=True)
            nc.scalar.dma_start(wconst[:, 2 * WQ:3 * WQ],
                                wconst_d.ap()[:, 2 * WQ:3 * WQ])
            dma_sbc(3, split=True)
            nc.scalar.dma_start(wconst[:, 3 * WQ:4 * WQ],
                                wconst_d.ap()[:, 3 * WQ:4 * WQ])
            nc.scalar.dma_start(wconst[:, 4 * WQ:], wconst_d.ap()[:, 4 * WQ:])
            nc.scalar.dma_start(cconst[:], cconst_d.ap())
            for q in range(4, NBLK):
                dma_sbc(q, split=True)

            # ---- warm-up: hoist all activation-table loads to t~0 and
            # bridge the PE through the DMA fill with dummy matmuls so the
            # first real matmul dispatches into a ramped p-state.
            scr = cp.tile([P, 192], BF16, tag="scr")
            nc.gpsimd.memset(scr[:], 0.0)
            scrf = cp.tile([P, 3], F32, tag="scrf")
            nc.gpsimd.memset(scrf[:], 0.0)
            nc.scalar.activation(scrf[:, 1:2], scrf[:, 0:1], Act.Silu)
            nc.scalar.copy(scrf[:, 1:2], scrf[:, 0:1])
            nc.scalar.activation(scrf[:, 2:3], scrf[:, 0:1], Act.Identity,
                                 bias=0.0)
            pscr = ps_h2.tile([P, 64], F32, tag="h2ps", name="pscr")
            for _ in range(WARMUP_MM):
                nc.tensor.matmul(pscr[:], scr[:, 0:128], scr[:, 128:192],
                                 start=True, stop=True)

            def stile(q):
                return pcs[q][:, 0:SCOLS]

            def atiles(q):
                return (pcs[q][:, SCOLS:SCOLS + ACOLS],
                        pcs[q][:, SCOLS + ACOLS:])

            w1f = wconst[:, OFF_W1:OFF_W1 + KT * P]
            w2f = wconst[:, OFF_W2:OFF_W2 + GA * P]
            sel = wconst[:, OFF_SEL:OFF_SEL + A]
            w3n = wconst[0:A, OFF_W3:OFF_W3 + A]
            w4n = wconst[0:A, OFF_W4:OFF_W4 + 1]
            b1col = cconst[:, 0:1]
            b3col = cconst[0:A, 1:2]
            b4one = cconst[0:1, 2:3]

            outsb = cp.tile([1, NPC], F32, tag="outsb")

            zas, zbs, sacts, p2s, h2ps = {}, {}, {}, {}, {}

            def z_form(q, split0=False):
                """DVE: tv 0..3 (za) and tv 4..6 (zb); Pool: tv 7."""
                st, (aa, ab) = stile(q), atiles(q)
                s_v = st.rearrange("p (tu n) -> p tu n", n=BLK)
                a_va = aa.rearrange("p (tv n) -> p tv n", n=BLK)
                a_vb = ab.rearrange("p (tv n) -> p tv n", n=BLK)
                za = za_p.tile([P, 16 * BLK], BF16, tag="za", name=f"za{q}")
                zb = zb_p.tile([P, 16 * BLK], BF16, tag="zb", name=f"zb{q}")
                za_v = za[:].rearrange("p (tv tu n) -> p tv tu n",
                                       tu=GS, n=BLK)
                if split0:
                    for h in range(2):
                        nc.vector.tensor_tensor(
                            za_v[:, 2 * h:2 * h + 2],
                            s_v.unsqueeze(1).broadcast_to([P, 2, GS, BLK]),
                            a_va[:, 2 * h:2 * h + 2].unsqueeze(2)
                            .broadcast_to([P, 2, GS, BLK]),
                            Alu.mult)
                else:
                    nc.vector.tensor_tensor(
                        za_v,
                        s_v.unsqueeze(1).broadcast_to([P, 4, GS, BLK]),
                        a_va.unsqueeze(2).broadcast_to([P, 4, GS, BLK]),
                        Alu.mult)
                nc.vector.tensor_tensor(
                    zb[:, 0:12 * BLK].rearrange(
                        "p (tv tu n) -> p tv tu n", tu=GS, n=BLK),
                    s_v.unsqueeze(1).broadcast_to([P, 3, GS, BLK]),
                    a_vb[:, 0:3, :].unsqueeze(2).broadcast_to(
                        [P, 3, GS, BLK]),
                    Alu.mult)
                nc.gpsimd.tensor_tensor(
                    zb[:, 12 * BLK:].rearrange(
                        "p (tu n) -> p tu n", n=BLK),
                    s_v,
                    a_vb[:, 3:4, :].broadcast_to([P, GS, BLK]),
                    Alu.mult)
                zas[q], zbs[q] = za, zb

            def s1_matmuls(q):
                acc = ps_acc.tile([P, BLK], F32, tag="acc", name=f"acc{q}")
                for t in range(KT):
                    z = zas[q] if t < KT // 2 else zbs[q]
                    zc = (t % (KT // 2)) * BLK
                    nc.tensor.matmul(
                        acc[:], w1f[:, t * P:(t + 1) * P],
                        z[:, zc:zc + BLK],
                        start=(t == 0), stop=(t == KT - 1))
                sact = sact_p.tile([P, BLK], BF16, tag="sact",
                                   name=f"sact{q}")
                nc.scalar.activation(sact[:], acc[:], Act.Silu, bias=b1col)
                sacts[q] = sact

            def t2t_and_mult(q):
                """Per half: 4 T2T matmuls, then the attr multiply.
                Route 'pool': ACT copy PSUM->SBUF bf16, Pool multiplies.
                Route 'dve2x': ACT copy, DVE multiplies at the 2x rate.
                Route 'dve': DVE multiplies straight from PSUM (no copy —
                used only at the drain to shorten the last chain)."""
                aa, ab = atiles(q)
                outs = []
                for half, asrc in enumerate((aa, ab)):
                    route = MULT_ROUTE[2 * q + half]
                    t2 = ps_t2.tile([P, ACOLS], F32, tag="t2",
                                    name=f"t2_{q}_{half}")
                    for bb in range(HGA):
                        nc.tensor.matmul(
                            t2[:, bb * BLK:(bb + 1) * BLK],
                            w2f[:, (half * HGA + bb) * P:
                                (half * HGA + bb + 1) * P],
                            sacts[q][:], start=True, stop=True)
                    p2 = p2_p.tile([P, ACOLS], BF16, tag="p2",
                                   name=f"p2_{q}_{half}")
                    if route == "dve":
                        nc.vector.tensor_tensor(p2[:], t2[:], asrc,
                                                Alu.mult)
                    else:
                        t2c = t2c_p.tile([P, ACOLS], BF16, tag="t2c",
                                         name=f"t2c_{q}_{half}")
                        nc.scalar.copy(t2c[:], t2[:])
                        eng = nc.gpsimd if route == "pool" else nc.vector
                        eng.tensor_tensor(p2[:], t2c[:], asrc, Alu.mult)
                    outs.append(p2)
                p2s[q] = tuple(outs)

            def sel_matmuls(q):
                """block q (one 128-node chunk): 8 accumulating matmuls
                with lhsT = Sel -> h2ps[q] [32w, n] (transposed)."""
                h2ps[q] = ps_h2.tile([A, GRP], F32, tag="h2ps",
                                     name=f"h2ps{q}")
                p2a, p2b = p2s[q]
                dst = h2ps[q][:]
                for b in range(GA):
                    src = p2a if b < HGA else p2b
                    c0 = (b % HGA) * BLK
                    nc.tensor.matmul(dst, sel, src[:, c0:c0 + BLK],
                                     start=(b == 0), stop=(b == GA - 1))

            def s34(g):
                """stages 3/4 for the 256-node group g, [feature, node]."""
                h2t = h2t_p.tile([A, GRP], BF16, tag="h2t", name=f"h2t{g}")
                nc.scalar.copy(h2t[:], h2ps[g][:])
                o3 = ps_acc.tile([A, GRP], F32, tag="acc", name=f"o3{g}")
                nc.tensor.matmul(o3[:], w3n, h2t[:], start=True, stop=True)
                h3t = h3t_p.tile([A, GRP], BF16, tag="h3t", name=f"h3t{g}")
                nc.scalar.activation(h3t[:], o3[:], Act.Silu, bias=b3col)
                o4 = ps_acc.tile([1, GRP], F32, tag="acc", name=f"o4{g}")
                nc.tensor.matmul(o4[:], w4n, h3t[:], start=True, stop=True)
                nc.scalar.activation(outsb[:, g * GRP:(g + 1) * GRP], o4[:],
                                     Act.Identity, bias=b4one)
                nc.sync.dma_start(out_d.ap()[:, g * GRP:(g + 1) * GRP],
                                  outsb[:, g * GRP:(g + 1) * GRP])

            # ---- pipelined emission ----
            z_form(0, split0=True)
            z_form(1)
            for i in range(NBLK):
                if i + 2 < NBLK:
                    z_form(i + 2)
                s1_matmuls(i)
                if i >= 1:
                    t2t_and_mult(i - 1)
                if i == NBLK - 1:
                    t2t_and_mult(i)
                if i >= 3:
                    sel_matmuls(i - 3)
                    s34(i - 3)
            # tail
            sel_matmuls(NBLK - 3)
            s34(NBLK - 3)
            sel_matmuls(NBLK - 2)
            s34(NBLK - 2)
            sel_matmuls(NBLK - 1)
            s34(NBLK - 1)

    nc.compile()
    return nc


def _get_nc():
    if "nc" not in _CACHE:
        _CACHE["nc"] = _build()
    return _CACHE["nc"]


def _prep_inputs(node_vec, node_embedding, W1s, b1s, W2, b2, W3, b3, W4, b4):
    f = np.float32
    inv = f(1.0 / 64.0)                      # 1/sqrt(128*32)
    s = np.ascontiguousarray(node_vec[:, :P]).astype(f)
    attr = np.asarray(node_embedding, f)

    pidx = np.arange(P)
    # k-tile tau = 4*tv + tu:  u = 32*tu + p%32,  v = 4*tv + p//32
    su_rows = A * np.arange(GS)[:, None] + (pidx % A)[None, :]    # [GS, P]
    av_rows = GS * np.arange(GA)[:, None] + (pidx // A)[None, :]  # [GA, P]

    wconst = np.zeros((P, FWC), BF)
    w1 = (np.asarray(W1s, f) * inv).astype(BF)           # [128u, 32v, 128w]
    for tv in range(GA):
        for tu in range(GS):
            t = GS * tv + tu
            wconst[:, OFF_W1 + t * P:OFF_W1 + (t + 1) * P] = \
                w1[su_rows[tu], av_rows[tv], :]
    # W2f_b[u, p2] = W2[u, 4b + p2//32, p2%32] * inv
    w2 = np.asarray(W2, f) * inv                         # [128u, 32v, 32w]
    for b in range(GA):
        wconst[:, OFF_W2 + b * P:OFF_W2 + (b + 1) * P] = \
            w2[:, GS * b + pidx // A, pidx % A].astype(BF)
    wconst[:, OFF_SEL:OFF_SEL + A] = \
        (pidx[:, None] % A == np.arange(A)[None, :]).astype(BF)
    w3n = np.asarray(W3, f) / np.sqrt(f(A))
    w4n = np.asarray(W4, f) / np.sqrt(f(A))
    wconst[0:A, OFF_W3:OFF_W3 + A] = w3n.astype(BF)
    wconst[0:A, OFF_W4:OFF_W4 + 1] = w4n.astype(BF)

    b3_eff = np.asarray(b3, f) + np.asarray(b2, f) @ w3n
    cconst = np.zeros((P, FCC), f)
    cconst[:, 0] = np.asarray(b1s, f)
    cconst[0:A, 1] = b3_eff
    cconst[0, 2] = np.asarray(b4, f).reshape(-1)[0]

    in_maps = []
    for core in range(NCORES):
        lo = core * NPC
        S = s[lo:lo + NPC].astype(BF)                     # [1024, 128]
        atb = attr[lo:lo + NPC].astype(BF)                # [1024, 32]

        sbc = np.empty((P, NBLK * PIECE), BF)
        for q in range(NBLK):
            Sb = S[q * BLK:(q + 1) * BLK]                 # [BLK, 128]
            Ab = atb[q * BLK:(q + 1) * BLK]               # [BLK, 32]
            base = q * PIECE
            for g in range(GS):
                sbc[:, base + g * BLK: base + (g + 1) * BLK] = Sb.T[su_rows[g]]
            base += GS * BLK
            for g in range(GA):
                sbc[:, base + g * BLK: base + (g + 1) * BLK] = Ab.T[av_rows[g]]

        in_maps.append(dict(wconst=wconst, cconst=cconst, sbc=sbc))
    return in_maps


def kernel(**inputs):
    global LAST_RESULT
    trace = bool(int(os.environ.get("KERNEL_TRACE", "0")))
    in_maps = _prep_inputs(
        inputs["node_vec"], inputs["node_embedding"],
        inputs["W1s"], inputs["b1s"], inputs["W2"], inputs["b2"],
        inputs["W3"], inputs["b3"], inputs["W4"], inputs["b4"],
    )
    nc = _get_nc()
    res = bass_utils.run_bass_kernel_spmd(
        nc, in_maps, core_ids=list(range(NCORES)), trace=trace)
    LAST_RESULT = res
    outs = [np.asarray(res.results[i]["out"]) for i in range(NCORES)]
    energy = np.concatenate([o.reshape(NPC) for o in outs]).reshape(N, 1)
    return energy.astype(np.float32)


# revision 7
# speedup vs baseline: 1.0917x; 1.0004x over previous
"""Trainium2 Bass kernel for nn_EquivariantScalar_viaTP — V3.3.

Reference computation (after dead-code elimination — the gate / l=1 / l=2
paths never reach the output):

    s      = node_vec[:, :128]                                  # [N, 128]
    attr   = node_embedding                                     # [N, 32]
    s_mid  = einsum('nu,nv,uvw->nw', s, attr, W1s) / 64 + b1s   # [N, 128]
    s_act  = silu(s_mid)
    h      = einsum('nu,nv,uvw->nw', s_act, attr, W2) / 64 + b2 # [N, 32]
    h      = silu(h @ (W3/sqrt(32)) + b3)                       # [N, 32]
    out    = h @ (W4/sqrt(32)) + b4                             # [N, 1]

Sharding: node dim N=8192 across 8 cores (1024 nodes each).

V3 design (engine-balanced, cost-model driven):
  Stage 1 — Z-outer-product form: s_mid^T[w,n] = sum_k W1f[k,w] Z[k,n],
  k=(u,v), 32 accumulating bf16 k-tile matmuls per 256-node block.
  k-tile tau=(tv,tu): partition p maps u = 32*tu + p%32, v = 4*tv + p//32.
  Z is formed elementwise from replicated sT / attrT tiles; broadcasts on
  the two OUTER free dims keep the 2x bf16 DVE mode.  The DVE forms
  tv 0..6 (two fused ops), the otherwise-idle Pool engine forms tv 7.
  silu(+b1) -> sact^T [u,n] bf16.

  Stage 2 transposed (T2T): per half-block, 4 matmuls with lhsT = W2f_b
  [128u, 128(v,w)] and rhs = sact^T produce T2T[(v,w), n] in PSUM
  (v(p2)=4b+p2//32, w(p2)=p2%32).  The attr multiply in2 is the SAME sbc
  attr tile stage 1 streams (its partition map matches v(p2) by
  construction).  GPSIMD cannot read PSUM (walrus rejects it), so blocks
  0..1 route ACT-copy(PSUM->SBUF bf16) -> Pool multiply, and blocks 2..3
  (DVE idle by then) multiply on the DVE straight from PSUM.
  The v-contraction is 8 accumulating PE matmuls per 128-node chunk with
  lhsT = a 0/1 selector Sel[p2,w] = (p2%32==w) and rhs = P2 — h2 lands
  TRANSPOSED [32w, n] in PSUM, so stages 3/4 need no PE transposes at
  all.  b2 folds into b3 (b3_eff = b3 + b2 @ W3/sqrt(A)).

  Stages 3/4 per 512-node group, all in the [feature, node] orientation:
  one ACT copy (PSUM->SBUF bf16), matmul lhsT=W3n, silu with per-
  partition bias b3, matmul lhsT=W4n, biased Identity copy -> out row
  [1, 512].  Output is [1, 1024] per core, node-ordered.

  The PE stream is emitted interleaved (S1(i) | T2T(i-1) | Sel(i-2)) and
  a tunable dummy-matmul warm-up bridges the PE through the DMA fill so
  real matmuls dispatch into a fully ramped p-state (the cost model
  charges p-state at dispatch; idle resets the ramp).
"""

import os

import numpy as np
import ml_dtypes

import concourse.bass as bass
import concourse.bacc as bacc
import concourse.mybir as mybir
from concourse.tile import TileContext
from concourse import bass_utils

N = 8192
P = 128          # partitions / MUL0
A = 32           # attr channels
NCORES = 8
NPC = N // NCORES          # 1024 nodes per core
NCHUNK = NPC // P          # 8 chunks per core
NBLK = 4                   # node blocks per core
BLK = NPC // NBLK          # 128 nodes
GS = 4                     # s-side tiles per block   (u = 32*tu + p%32)
GA = 8                     # attr-side tiles per block (v = 4*tv + p//32)
KT = GS * GA               # 32 k-tiles
HGA = GA // 2              # 4 attr tiles per half
SCOLS = GS * BLK           # 1024 s cols per block
ACOLS = HGA * BLK          # 1024 attr cols per half
PIECE = SCOLS + 2 * ACOLS  # 3072
NGRP = 4                   # stage-3/4 groups
GRP = NPC // NGRP          # 256 nodes per group
# per-half mult routing: 16 halves (2 per block)
MULT_ROUTE = ["pool", "dve2x"] * 3 + ["dve", "dve"]
WARMUP_MM = 66

F32 = mybir.dt.float32
BF16 = mybir.dt.bfloat16
BF = ml_dtypes.bfloat16

# wconst (bf16) column offsets
OFF_W1 = 0
OFF_W2 = OFF_W1 + KT * P          # 4096
OFF_SEL = OFF_W2 + GA * P         # 5120
OFF_W3 = OFF_SEL + A              # w3n on partitions 0..31
OFF_W4 = OFF_W3 + A               # w4n on partitions 0..31
FWC = OFF_W4 + 1                  # 5185
FCC = 3                           # cconst (f32): b1col | b3col | b4col


_CACHE = {}
LAST_RESULT = None         # test harness reads exec_time_ns from here


def _build():
    nc = bacc.Bacc(trn_type="TRN2", target_bir_lowering=False, debug=False)

    wconst_d = nc.dram_tensor("wconst", [P, FWC], BF16, kind="ExternalInput")
    cconst_d = nc.dram_tensor("cconst", [P, FCC], F32, kind="ExternalInput")
    sbc_d = nc.dram_tensor("sbc", [P, NBLK * PIECE], BF16,
                           kind="ExternalInput")
    out_d = nc.dram_tensor("out", [1, NPC], F32, kind="ExternalOutput")

    Alu = mybir.AluOpType
    Act = mybir.ActivationFunctionType

    with TileContext(nc) as tc:
        with (
            tc.tile_pool(name="const", bufs=1) as cp,
            tc.tile_pool(name="pc", bufs=8) as pc_p,
            tc.tile_pool(name="za", bufs=3) as za_p,
            tc.tile_pool(name="zb", bufs=3) as zb_p,
            tc.tile_pool(name="sact", bufs=3) as sact_p,
            tc.tile_pool(name="t2c", bufs=3) as t2c_p,
            tc.tile_pool(name="p2", bufs=8) as p2_p,
            tc.tile_pool(name="h2t", bufs=2) as h2t_p,
            tc.tile_pool(name="h3t", bufs=2) as h3t_p,
            tc.tile_pool(name="psacc", bufs=2, space="PSUM") as ps_acc,
            tc.tile_pool(name="pst2", bufs=2, space="PSUM") as ps_t2,
            tc.tile_pool(name="psh2", bufs=2, space="PSUM") as ps_h2,
        ):
            wconst = cp.tile([P, FWC], BF16, tag="wconst")
            cconst = cp.tile([P, FCC], F32, tag="cconst")
            pcs = {}

            def dma_sbc(q, split=False):
                pc = pc_p.tile([P, PIECE], BF16, tag="pc", name=f"pc{q}")
                if split:
                    nc.sync.dma_start(
                        pc[:, 0:SCOLS + ACOLS],
                        sbc_d.ap()[:, q * PIECE:q * PIECE + SCOLS + ACOLS])
                    nc.sync.dma_start(
                        pc[:, SCOLS + ACOLS:],
                        sbc_d.ap()[:, q * PIECE + SCOLS + ACOLS:
                                   (q + 1) * PIECE])
                else:
                    nc.sync.dma_start(
                        pc[:], sbc_d.ap()[:, q * PIECE:(q + 1) * PIECE])
                pcs[q] = pc

            # DMA dispatch order: SP ring carries sbc pieces; ACT ring
            # carries w1 (quarters, so S1(0) unblocks progressively), the
            # rest of wconst, and cconst.  Dispatch interleaves so the
            # shared wire serves the z-critical pieces first.
            WQ = KT * P // 4
            dma_sbc(0, split=True)
            nc.scalar.dma_start(wconst[:, 0:WQ], wconst_d.ap()[:, 0:WQ])
            dma_sbc(1, split=True)
            nc.scalar.dma_start(wconst[:, WQ:2 * WQ],
                                wconst_d.ap()[:, WQ:2 * WQ])
            dma_sbc(2, split=True)
            nc.scalar.dma_start(wconst[:, 2 * WQ:3 * WQ],
                                wconst_d.ap()[:, 2 * WQ:3 * WQ])
            dma_sbc(3, split=True)
            nc.scalar.dma_start(wconst[:, 3 * WQ:4 * WQ],
                                wconst_d.ap()[:, 3 * WQ:4 * WQ])
            nc.scalar.dma_start(wconst[:, 4 * WQ:], wconst_d.ap()[:, 4 * WQ:])
            nc.scalar.dma_start(cconst[:], cconst_d.ap())
            for q in range(4, NBLK):
                dma_sbc(q, split=True)

            # ---- warm-up: hoist all activation-table loads to t~0 and
            # bridge the PE through the DMA fill with dummy matmuls so the
            # first real matmul dispatches into a ramped p-state.
            scr = cp.tile([P, 192], BF16, tag="scr")
            nc.gpsimd.memset(scr[:], 0.0)
            scrf = cp.tile([P, 3], F32, tag="scrf")
            nc.gpsimd.memset(scrf[:], 0.0)
            nc.scalar.activation(scrf[:, 1:2], scrf[:, 0:1], Act.Silu)
            nc.scalar.copy(scrf[:, 1:2], scrf[:, 0:1])
            nc.scalar.activation(scrf[:, 2:3], scrf[:, 0:1], Act.Identity,
                                 bias=0.0)
            pscr = ps_h2.tile([P, 64], F32, tag="h2ps", name="pscr")
            for _ in range(WARMUP_MM):
                nc.tensor.matmul(pscr[:], scr[:, 0:128], scr[:, 128:192],
                                 start=True, stop=True)

            def stile(q):
                return pcs[q][:, 0:SCOLS]

            def atiles(q):
                return (pcs[q][:, SCOLS:SCOLS + ACOLS],
                        pcs[q][:, SCOLS + ACOLS:])

            w1f = wconst[:, OFF_W1:OFF_W1 + KT * P]
            w2f = wconst[:, OFF_W2:OFF_W2 + GA * P]
            sel = wconst[:, OFF_SEL:OFF_SEL + A]
            w3n = wconst[0:A, OFF_W3:OFF_W3 + A]
            w4n = wconst[0:A, OFF_W4:OFF_W4 + 1]
            b1col = cconst[:, 0:1]
            b3col = cconst[0:A, 1:2]
            b4one = cconst[0:1, 2:3]

            outsb = cp.tile([1, NPC], F32, tag="outsb")

            zas, zbs, sacts, p2s, h2ps = {}, {}, {}, {}, {}

            def z_form(q, split0=False):
                """DVE: tv 0..3 (za) and tv 4..6 (zb); Pool: tv 7."""
                st, (aa, ab) = stile(q), atiles(q)
                s_v = st.rearrange("p (tu n) -> p tu n", n=BLK)
                a_va = aa.rearrange("p (tv n) -> p tv n", n=BLK)
                a_vb = ab.rearrange("p (tv n) -> p tv n", n=BLK)
                za = za_p.tile([P, 16 * BLK], BF16, tag="za", name=f"za{q}")
                zb = zb_p.tile([P, 16 * BLK], BF16, tag="zb", name=f"zb{q}")
                za_v = za[:].rearrange("p (tv tu n) -> p tv tu n",
                                       tu=GS, n=BLK)
                if split0:
                    for h in range(2):
                        nc.vector.tensor_tensor(
                            za_v[:, 2 * h:2 * h + 2],
                            s_v.unsqueeze(1).broadcast_to([P, 2, GS, BLK]),
                            a_va[:, 2 * h:2 * h + 2].unsqueeze(2)
                            .broadcast_to([P, 2, GS, BLK]),
                            Alu.mult)
                else:
                    nc.vector.tensor_tensor(
                        za_v,
                        s_v.unsqueeze(1).broadcast_to([P, 4, GS, BLK]),
                        a_va.unsqueeze(2).broadcast_to([P, 4, GS, BLK]),
                        Alu.mult)
                nc.vector.tensor_tensor(
                    zb[:, 0:12 * BLK].rearrange(
                        "p (tv tu n) -> p tv tu n", tu=GS, n=BLK),
                    s_v.unsqueeze(1).broadcast_to([P, 3, GS, BLK]),
                    a_vb[:, 0:3, :].unsqueeze(2).broadcast_to(
                        [P, 3, GS, BLK]),
                    Alu.mult)
                nc.gpsimd.tensor_tensor(
                    zb[:, 12 * BLK:].rearrange(
                        "p (tu n) -> p tu n", n=BLK),
                    s_v,
                    a_vb[:, 3:4, :].broadcast_to([P, GS, BLK]),
                    Alu.mult)
                zas[q], zbs[q] = za, zb

            def s1_matmuls(q):
                acc = ps_acc.tile([P, BLK], F32, tag="acc", name=f"acc{q}")
                for t in range(KT):
                    z = zas[q] if t < KT // 2 else zbs[q]
                    zc = (t % (KT // 2)) * BLK
                    nc.tensor.matmul(
                        acc[:], w1f[:, t * P:(t + 1) * P],
                        z[:, zc:zc + BLK],
                        start=(t == 0), stop=(t == KT - 1))
                sact = sact_p.tile([P, BLK], BF16, tag="sact",
                                   name=f"sact{q}")
                nc.scalar.activation(sact[:], acc[:], Act.Silu, bias=b1col)
                sacts[q] = sact

            def t2t_and_mult(q):
                """Per half: 4 T2T matmuls, then the attr multiply.
                Route 'pool': ACT copy PSUM->SBUF bf16, Pool multiplies.
                Route 'dve2x': ACT copy, DVE multiplies at the 2x rate.
                Route 'dve': DVE multiplies straight from PSUM (no copy —
                used only at the drain to shorten the last chain)."""
                aa, ab = atiles(q)
                outs = []
                for half, asrc in enumerate((aa, ab)):
                    route = MULT_ROUTE[2 * q + half]
                    t2 = ps_t2.tile([P, ACOLS], F32, tag="t2",
                                    name=f"t2_{q}_{half}")
                    for bb in range(HGA):
                        nc.tensor.matmul(
                            t2[:, bb * BLK:(bb + 1) * BLK],
                            w2f[:, (half * HGA + bb) * P:
                                (half * HGA + bb + 1) * P],
                            sacts[q][:], start=True, stop=True)
                    p2 = p2_p.tile([P, ACOLS], BF16, tag="p2",
                                   name=f"p2_{q}_{half}")
                    if route == "dve":
                        nc.vector.tensor_tensor(p2[:], t2[:], asrc,
                                                Alu.mult)
                    else:
                        t2c = t2c_p.tile([P, ACOLS], BF16, tag="t2c",
                                         name=f"t2c_{q}_{half}")
                        nc.scalar.copy(t2c[:], t2[:])
                        eng = nc.gpsimd if route == "pool" else nc.vector
                        eng.tensor_tensor(p2[:], t2c[:], asrc, Alu.mult)
                    outs.append(p2)
                p2s[q] = tuple(outs)

            def sel_matmuls(q):
                """block q (one 128-node chunk): 8 accumulating matmuls
                with lhsT = Sel -> h2ps[q] [32w, n] (transposed)."""
                h2ps[q] = ps_h2.tile([A, GRP], F32, tag="h2ps",
                                     name=f"h2ps{q}")
                p2a, p2b = p2s[q]
                dst = h2ps[q][:]
                for b in range(GA):
                    src = p2a if b < HGA else p2b
                    c0 = (b % HGA) * BLK
                    nc.tensor.matmul(dst, sel, src[:, c0:c0 + BLK],
                                     start=(b == 0), stop=(b == GA - 1))

            def s34(g):
                """stages 3/4 for the 256-node group g, [feature, node]."""
                h2t = h2t_p.tile([A, GRP], BF16, tag="h2t", name=f"h2t{g}")
                nc.scalar.copy(h2t[:], h2ps[g][:])
                o3 = ps_acc.tile([A, GRP], F32, tag="acc", name=f"o3{g}")
                nc.tensor.matmul(o3[:], w3n, h2t[:], start=True, stop=True)
                h3t = h3t_p.tile([A, GRP], BF16, tag="h3t", name=f"h3t{g}")
                nc.scalar.activation(h3t[:], o3[:], Act.Silu, bias=b3col)
                o4 = ps_acc.tile([1, GRP], F32, tag="acc", name=f"o4{g}")
                nc.tensor.matmul(o4[:], w4n, h3t[:], start=True, stop=True)
                nc.scalar.activation(outsb[:, g * GRP:(g + 1) * GRP], o4[:],
                                     Act.Identity, bias=b4one)
                nc.sync.dma_start(out_d.ap()[:, g * GRP:(g + 1) * GRP],
                                  outsb[:, g * GRP:(g + 1) * GRP])

            # ---- pipelined emission ----
            z_form(0, split0=True)
            z_form(1)
            for i in range(NBLK):
                if i + 2 < NBLK:
                    z_form(i + 2)
                s1_matmuls(i)
                if i >= 1:
                    t2t_and_mult(i - 1)
                if i == NBLK - 1:
                    t2t_and_mult(i)
                if i >= 3:
                    sel_matmuls(i - 3)
                    s34(i - 3)
            # tail
            sel_matmuls(NBLK - 3)
            s34(NBLK - 3)
            sel_matmuls(NBLK - 2)
            s34(NBLK - 2)
            sel_matmuls(NBLK - 1)
            s34(NBLK - 1)

    nc.compile()
    return nc


def _get_nc():
    if "nc" not in _CACHE:
        _CACHE["nc"] = _build()
    return _CACHE["nc"]


def _prep_inputs(node_vec, node_embedding, W1s, b1s, W2, b2, W3, b3, W4, b4):
    f = np.float32
    inv = f(1.0 / 64.0)                      # 1/sqrt(128*32)
    s = np.ascontiguousarray(node_vec[:, :P]).astype(f)
    attr = np.asarray(node_embedding, f)

    pidx = np.arange(P)
    # k-tile tau = 4*tv + tu:  u = 32*tu + p%32,  v = 4*tv + p//32
    su_rows = A * np.arange(GS)[:, None] + (pidx % A)[None, :]    # [GS, P]
    av_rows = GS * np.arange(GA)[:, None] + (pidx // A)[None, :]  # [GA, P]

    wconst = np.zeros((P, FWC), BF)
    w1 = (np.asarray(W1s, f) * inv).astype(BF)           # [128u, 32v, 128w]
    for tv in range(GA):
        for tu in range(GS):
            t = GS * tv + tu
            wconst[:, OFF_W1 + t * P:OFF_W1 + (t + 1) * P] = \
                w1[su_rows[tu], av_rows[tv], :]
    # W2f_b[u, p2] = W2[u, 4b + p2//32, p2%32] * inv
    w2 = np.asarray(W2, f) * inv                         # [128u, 32v, 32w]
    for b in range(GA):
        wconst[:, OFF_W2 + b * P:OFF_W2 + (b + 1) * P] = \
            w2[:, GS * b + pidx // A, pidx % A].astype(BF)
    wconst[:, OFF_SEL:OFF_SEL + A] = \
        (pidx[:, None] % A == np.arange(A)[None, :]).astype(BF)
    w3n = np.asarray(W3, f) / np.sqrt(f(A))
    w4n = np.asarray(W4, f) / np.sqrt(f(A))
    wconst[0:A, OFF_W3:OFF_W3 + A] = w3n.astype(BF)
    wconst[0:A, OFF_W4:OFF_W4 + 1] = w4n.astype(BF)

    b3_eff = np.asarray(b3, f) + np.asarray(b2, f) @ w3n
    cconst = np.zeros((P, FCC), f)
    cconst[:, 0] = np.asarray(b1s, f)
    cconst[0:A, 1] = b3_eff
    cconst[0, 2] = np.asarray(b4, f).reshape(-1)[0]

    in_maps = []
    for core in range(NCORES):
        lo = core * NPC
        S = s[lo:lo + NPC].astype(BF)                     # [1024, 128]
        atb = attr[lo:lo + NPC].astype(BF)                # [1024, 32]

        sbc = np.empty((P, NBLK * PIECE), BF)
        for q in range(NBLK):
            Sb = S[q * BLK:(q + 1) * BLK]                 # [BLK, 128]
            Ab = atb[q * BLK:(q + 1) * BLK]               # [BLK, 32]
            base = q * PIECE
            for g in range(GS):
                sbc[:, base + g * BLK: base + (g + 1) * BLK] = Sb.T[su_rows[g]]
            base += GS * BLK
            for g in range(GA):
                sbc[:, base + g * BLK: base + (g + 1) * BLK] = Ab.T[av_rows[g]]

        in_maps.append(dict(wconst=wconst, cconst=cconst, sbc=sbc))
    return in_maps


def kernel(**inputs):
    global LAST_RESULT
    trace = bool(int(os.environ.get("KERNEL_TRACE", "0")))
    in_maps = _prep_inputs(
        inputs["node_vec"], inputs["node_embedding"],
        inputs["W1s"], inputs["b1s"], inputs["W2"], inputs["b2"],
        inputs["W3"], inputs["b3"], inputs["W4"], inputs["b4"],
    )
    nc = _get_nc()
    res = bass_utils.run_bass_kernel_spmd(
        nc, in_maps, core_ids=list(range(NCORES)), trace=trace)
    LAST_RESULT = res
    outs = [np.asarray(res.results[i]["out"]) for i in range(NCORES)]
    energy = np.concatenate([o.reshape(NPC) for o in outs]).reshape(N, 1)
    return energy.astype(np.float32)
